# revision 13
# baseline (speedup 1.0000x reference)
"""GAT layer (nn_GATLayer) on 8 Trainium2 NeuronCores via Bass/Tile.

Strategy (dst-partitioned; degree-aligned slots, softmax fully local per core):
  - Core k owns dst nodes [k*6250, (k+1)*6250). Each owned node is pinned to a
    (block, partition) slot; ALL of its in-edges occupy that partition across
    the block's chunks. Segment-softmax then needs no scatter at all: the
    per-node sums are free-axis reductions, and the dst attention term ad is a
    per-partition scalar broadcast.
  - Phase A (replicated): zaug[row] = [z bf16 x64 | as f32 x4 | pad] (256B rows)
    for ALL nodes via one matmul with folded weights [W.T | W.T@A1blk].
    Rows are partition-major (row = p*391 + s) so stores are contiguous.
    Two reserved rows (0 and 50047) get as = -60 patched in: pad slots gather
    them and contribute exp(leaky(-60+ad)) ~ 1e-5 to denom and 0 to num (z=0).
  - Phase A0: ad = hT_own @ (W.T@A2blk) for own nodes in (block, pos) order;
    stays SBUF-resident [128, 49, 4].
  - Edge phase per block: dma_gather of zaug rows by src (two overlapping
    int16 views: A = rows [0, 32768), B = rows [17280, 50048); per-node lo/hi
    edge split chosen on host), e = as + ad, ex = max(exp(e), exp(0.2e)),
    rhs = [ex*zs | ex], then one reduce_sum over chunks -> [num | denom];
    out = num * recip(denom), accumulated in SBUF, single store at the end.
  - Gather calls span several blocks (SWDGE ring enlarged to 4096 descs) to
    amortize the per-call desc-gen overhead on Pool.

All index/layout prep (row permutation placing high-out-degree nodes in the
overlapping view region, per-core 2D block packing, per-block KLO/KHI chunk
budgets uniformized across cores so one program serves all 8) is done on host.
"""

import numpy as np

N_NODES = 50000
N_EDGES = 800000
IN_FEATS = 128
OUT_FEATS = 16
NUM_HEADS = 4
ALPHA = 0.2
HF = NUM_HEADS * OUT_FEATS  # 64

N_CORES = 8
P = 128
NODES_PER_CORE = N_NODES // N_CORES     # 6250
BLOCKS = 49                              # ceil(6250/128)
NODE_PAD = BLOCKS * P                    # 6272
NCHUNK = 391                             # zaug chunks; 128*391 = 50048 rows
N_NODES_PAD = P * NCHUNK                 # 50048
VIEW = 32768                             # int16 gather view size
HIB = N_NODES_PAD - VIEW                 # 17280 = base of view B
PAD_A_ROW = 0                            # reserved pad row in view A (p0, s0)
PAD_B_ROW = 96 * NCHUNK                  # reserved pad row in view B (p96, s0)
DMA_SCRATCH = 65536                      # SWDGE ring: 4096 descriptors
CALL_CHUNKS = 8                          # max chunks (128 idx each) per gather
TILE_CHUNKS = 24                         # max chunks per stream per zs tile


def _wrap16(vals):
    # gather idx layout: stream position i -> idx tile [16, n/16] at
    # [i%16, i//16]; rows replicated to 128 partitions.
    n = vals.shape[-1]
    w = vals.reshape(n // 16, 16).T                    # [16, n/16]
    return np.tile(w, (8, 1))                          # [128, n/16]


def _plan_groups(klos, khis):
    """Greedy grouping of consecutive blocks into zs-tile groups with
    sum(KLO) <= TILE_CHUNKS and sum(KHI) <= TILE_CHUNKS (single blocks may
    exceed the cap; their gathers are split into CALL_CHUNKS-sized calls)."""
    groups = []
    cur = []
    sa = sb = 0
    for b in range(len(klos)):
        ka, kb = klos[b], khis[b]
        if cur and (sa + ka > TILE_CHUNKS or sb + kb > TILE_CHUNKS):
            groups.append(cur)
            cur, sa, sb = [], 0, 0
        cur.append(b)
        sa += ka
        sb += kb
    if cur:
        groups.append(cur)
    return groups


def _build_host_data(h, src, dst, W, A):
    import ml_dtypes

    src = np.asarray(src)
    dst = np.asarray(dst)
    W = np.asarray(W, dtype=np.float32)
    A = np.asarray(A, dtype=np.float32)
    h = np.asarray(h, dtype=np.float32)

    # folded weights
    A1blk = np.zeros((HF, NUM_HEADS), dtype=np.float32)
    A2blk = np.zeros((HF, NUM_HEADS), dtype=np.float32)
    for hd in range(NUM_HEADS):
        A1blk[hd * OUT_FEATS:(hd + 1) * OUT_FEATS, hd] = A[hd, :OUT_FEATS]
        A2blk[hd * OUT_FEATS:(hd + 1) * OUT_FEATS, hd] = A[hd, OUT_FEATS:]
    WT = np.ascontiguousarray(W.T)                                  # [128, 64]
    wcat = np.concatenate([WT, WT @ A1blk], axis=1).astype(ml_dtypes.bfloat16)
    wad = (WT @ A2blk).astype(ml_dtypes.bfloat16)                   # [128, 4]

    # global row permutation: high-out-degree nodes -> overlap rows
    # [HIB, VIEW); rows 0 and 50047 reserved for pad targets.
    outdeg = np.bincount(src, minlength=N_NODES)
    nodes_by_heat = np.argsort(-outdeg, kind="stable")
    ov_rows = np.arange(HIB, VIEW)
    rest_hi = np.arange(VIEW, N_NODES_PAD)
    rest = np.concatenate(
        [np.arange(1, HIB), rest_hi[rest_hi != PAD_B_ROW]])
    perm = np.empty(N_NODES, dtype=np.int64)
    perm[nodes_by_heat[:len(ov_rows)]] = ov_rows
    perm[nodes_by_heat[len(ov_rows):]] = rest[:N_NODES - len(ov_rows)]

    # hT column for row r: phase A chunk s partition p -> row p*391 + s,
    # processed from hT col s*128 + p.
    hT = np.zeros((P, N_NODES_PAD), dtype=ml_dtypes.bfloat16)
    cols = (perm % NCHUNK) * P + perm // NCHUNK
    hT[:, cols] = h.T.astype(ml_dtypes.bfloat16)

    # per-core edge prep
    order = np.argsort(dst, kind="stable")
    dst_s = dst[order]
    rows_s = perm[src[order]]
    core_begin = np.searchsorted(
        dst_s, np.arange(0, N_NODES + 1, NODES_PER_CORE))

    cores = []
    for k in range(N_CORES):
        lo_e, hi_e = core_begin[k], core_begin[k + 1]
        cd = dst_s[lo_e:hi_e] - k * NODES_PER_CORE
        rw = rows_s[lo_e:hi_e]
        is_a = rw < HIB                      # A-only
        is_b = rw >= VIEW                    # B-only
        is_f = ~is_a & ~is_b                 # flexible
        a = np.bincount(cd[is_a], minlength=NODES_PER_CORE)
        c = np.bincount(cd[is_b], minlength=NODES_PER_CORE)
        f = np.bincount(cd[is_f], minlength=NODES_PER_CORE)
        T = a + c + f
        node_order = np.lexsort((a, -T // 2))
        # node -> (block, pos)
        node_block = np.empty(NODES_PER_CORE, dtype=np.int64)
        node_pos = np.empty(NODES_PER_CORE, dtype=np.int64)
        node_block[node_order] = np.arange(NODES_PER_CORE) // P
        node_pos[node_order] = np.arange(NODES_PER_CORE) % P
        # per-block optimal (KLO, KHI)
        klo = np.zeros(BLOCKS, dtype=np.int64)
        khi = np.zeros(BLOCKS, dtype=np.int64)
        for b in range(BLOCKS):
            blk = node_order[b * P:(b + 1) * P]
            ab, cb_, fb, Tb = a[blk], c[blk], f[blk], T[blk]
            best = None
            for KLO in range(int(ab.max()), int(Tb.max()) + 1):
                KHI = int(np.maximum(cb_, Tb - np.minimum(KLO, ab + fb)).max())
                if best is None or KLO + KHI < best[0]:
                    best = (KLO + KHI, KLO, KHI)
                if KHI == int(cb_.max()):
                    break
            klo[b], khi[b] = best[1], best[2]
        cores.append(dict(cd=cd, rw=rw, a=a, c=c, f=f, T=T,
                          node_block=node_block, node_pos=node_pos,
                          klo=klo, khi=khi))

    # uniform per-block chunk budgets across cores (one program, 8 cores)
    KLOs = np.max([co["klo"] for co in cores], axis=0)
    KHIs = np.max([co["khi"] for co in cores], axis=0)
    groups = _plan_groups(KLOs, KHIs)
    LA = int(KLOs.sum()) * P
    LB = int(KHIs.sum()) * P
    offA = np.concatenate([[0], np.cumsum(KLOs)])    # chunk offsets per block
    offB = np.concatenate([[0], np.cumsum(KHIs)])

    in_maps = []
    unpack_maps = []
    for k in range(N_CORES):
        co = cores[k]
        cd, rw = co["cd"], co["rw"]
        a, f, T = co["a"], co["f"], co["T"]
        node_block, node_pos = co["node_block"], co["node_pos"]
        # per-node lo count: L = max(a, T - KHI_block)
        KHI_n = KHIs[node_block]
        L = np.maximum(a, T - KHI_n)

        # sort edges by (node, flexibility-class) so each node's edge list is
        # [A-only..., flex..., B-only...]; first L edges -> stream A.
        cls = np.where(rw < HIB, 0, np.where(rw < VIEW, 1, 2))
        eo = np.lexsort((cls, cd))
        cd_o, rw_o = cd[eo], rw[eo]
        starts = np.searchsorted(cd_o, np.arange(NODES_PER_CORE + 1))
        rank = np.arange(len(cd_o)) - starts[cd_o]          # rank within node
        to_a = rank < L[cd_o]

        gA = np.full((LA // P, P), PAD_A_ROW, dtype=np.int16)
        gB = np.full((LB // P, P), PAD_B_ROW - HIB, dtype=np.int16)
        # slot chunk = offX[block] + rank (A) or rank - L (B)
        blk_e = node_block[cd_o]
        pos_e = node_pos[cd_o]
        ca = offA[blk_e] + rank
        cb_ = offB[blk_e] + rank - L[cd_o]
        gA[ca[to_a], pos_e[to_a]] = rw_o[to_a].astype(np.int16)
        gB[cb_[~to_a], pos_e[~to_a]] = (rw_o[~to_a] - HIB).astype(np.int16)

        # wrap16 per call group
        gAw, gBw = [], []
        for g in groups:
            b0, b1 = g[0], g[-1] + 1
            gAw.append(_wrap16(gA[offA[b0]:offA[b1]].reshape(-1)))
            gBw.append(_wrap16(gB[offB[b0]:offB[b1]].reshape(-1)))
        gAw = np.ascontiguousarray(np.concatenate(gAw, axis=1))
        gBw = np.ascontiguousarray(np.concatenate(gBw, axis=1))

        # hT_own: col b*128 + pos = h[node]
        hT_own = np.zeros((P, NODE_PAD), dtype=ml_dtypes.bfloat16)
        own = np.arange(k * NODES_PER_CORE, (k + 1) * NODES_PER_CORE)
        hT_own[:, node_block * P + node_pos] = h[own].T.astype(
            ml_dtypes.bfloat16)

        in_maps.append({
            "hT": hT,
            "hT_own": hT_own,
            "wcat": np.ascontiguousarray(wcat),
            "wad": np.ascontiguousarray(wad),
            "gidxA": gAw,
            "gidxB": gBw,
        })
        # outO row for node (block, pos) = pos*BLOCKS + block
        unpack_maps.append(node_pos * BLOCKS + node_block)

    return in_maps, (KLOs, KHIs, groups), unpack_maps


def _build_program(plan):
    import concourse.bacc as bacc
    import concourse.tile as tile
    import concourse.mybir as mybir

    KLOs, KHIs, groups = plan
    LA = int(KLOs.sum()) * P
    LB = int(KHIs.sum()) * P
    f32 = mybir.dt.float32
    bf16 = mybir.dt.bfloat16
    i16 = mybir.dt.int16

    import os as _os
    _simclean = _os.environ.get("SIM_CLEAN", "0") == "1"
    nc = bacc.Bacc("TRN2", target_bir_lowering=False, debug=False,
                   dynamic_dma_scratch_size=DMA_SCRATCH)

    hT = nc.dram_tensor("hT", [P, N_NODES_PAD], bf16, kind="ExternalInput")
    hT_own = nc.dram_tensor("hT_own", [P, NODE_PAD], bf16, kind="ExternalInput")
    wcat_d = nc.dram_tensor("wcat", [P, 68], bf16, kind="ExternalInput")
    wad_d = nc.dram_tensor("wad", [P, 4], bf16, kind="ExternalInput")
    gidxA = nc.dram_tensor("gidxA", [P, LA // 16], i16, kind="ExternalInput")
    gidxB = nc.dram_tensor("gidxB", [P, LB // 16], i16, kind="ExternalInput")

    zaug = nc.dram_tensor("zaug", [N_NODES_PAD, 64], f32)
    outO = nc.dram_tensor("outO", [NODE_PAD, HF], f32, kind="ExternalOutput")

    SC = 4                        # chunks per PSUM tile
    SC2 = 8                       # chunks per load/store superchunk

    with tile.TileContext(nc) as tc:
        with (
            tc.tile_pool(name="const", bufs=1) as cpool,
            tc.tile_pool(name="pa", bufs=8) as pa,
            tc.tile_pool(name="papsum", bufs=4, space="PSUM") as papsum,
            tc.tile_pool(name="adpsum", bufs=1, space="PSUM") as adpsum,
            tc.tile_pool(name="epA", bufs=2) as epA,
            tc.tile_pool(name="epB", bufs=2) as epB,
            tc.tile_pool(name="ep", bufs=3) as ep,
            tc.tile_pool(name="fp", bufs=3) as fp,
        ):
            wcat_t = cpool.tile([P, 68], bf16)
            nc.sync.dma_start(out=wcat_t[:], in_=wcat_d[:])
            wad_t = cpool.tile([P, 4], bf16)
            nc.sync.dma_start(out=wad_t[:], in_=wad_d[:])
            ho_t = cpool.tile([P, NODE_PAD], bf16)
            nc.sync.dma_start(out=ho_t[:], in_=hT_own[:])
            gA_t = cpool.tile([P, LA // 16], i16)
            nc.sync.dma_start(out=gA_t[:], in_=gidxA[:])
            gB_t = cpool.tile([P, LB // 16], i16)
            nc.sync.dma_start(out=gB_t[:], in_=gidxB[:])

            # ---------------- Phase A0: ad for own nodes (SBUF resident) ----
            adp = adpsum.tile([P, BLOCKS, 4], f32)
            for b in range(BLOCKS):
                nc.tensor.matmul(
                    out=adp[:, b, :],
                    lhsT=ho_t[:, b * P:(b + 1) * P],
                    rhs=wad_t[:],
                    start=True, stop=True,
                )
            adall = cpool.tile([P, BLOCKS, 4], f32)
            nc.scalar.copy(out=adall[:], in_=adp[:])

            # ---------------- Phase A: zaug for all nodes -------------------
            n_sc = NCHUNK // SC2
            all_scs = [(s * SC2, SC2) for s in range(n_sc)]
            if NCHUNK % SC2:
                all_scs.append((n_sc * SC2, NCHUNK % SC2))
            z3 = zaug[:].rearrange("(p s) e -> p s e", s=NCHUNK)
            for s0, nsub in all_scs:
                hsl = pa.tile([P, SC2 * P], bf16, tag="hsl")
                nc.sync.dma_start(
                    out=hsl[:, :nsub * P],
                    in_=hT[:, s0 * P:(s0 + nsub) * P])
                zst = pa.tile([P, SC2, 64], f32, tag="zst")
                if _simclean:
                    nc.scalar.memzero(zst[:])
                zbf = zst[:].bitcast(mybir.dt.bfloat16)
                for g0 in range(0, nsub, SC):
                    g1 = min(g0 + SC, nsub)
                    zp = papsum.tile([P, SC, 68], f32, tag="zp")
                    for j in range(g0, g1):
                        nc.tensor.matmul(
                            out=zp[:, j - g0, :],
                            lhsT=hsl[:, j * P:(j + 1) * P],
                            rhs=wcat_t[:],
                            start=True, stop=True,
                        )
                    nc.scalar.copy(out=zbf[:, g0:g1, 0:64],
                                   in_=zp[:, :g1 - g0, 0:64])
                    nc.scalar.copy(out=zst[:, g0:g1, 32:36],
                                   in_=zp[:, :g1 - g0, 64:68])
                # overwrite as = -60 on the reserved pad rows (p, s) =
                # (0, 0) and (96, 0) before the store
                if s0 == 0:
                    nc.vector.memset(zst[0:1, 0, 32:36], -60.0)
                    nc.vector.memset(zst[96:97, 0, 32:36], -60.0)
                nc.sync.dma_start(out=z3[:, s0:s0 + nsub, :],
                                  in_=zst[:, :nsub, :])

            # ---------------- Edge phase ------------------------------------
            viewA = zaug[0:VIEW, :]
            viewB = zaug[HIB:N_NODES_PAD, :]
            outS = cpool.tile([P, BLOCKS, HF], f32)
            offA = np.concatenate([[0], np.cumsum(KLOs)])
            offB = np.concatenate([[0], np.cumsum(KHIs)])
            o16A = o16B = 0
            for g in groups:
                b0, b1 = g[0], g[-1] + 1
                ga_ch = int(offA[b1] - offA[b0])
                gb_ch = int(offB[b1] - offB[b0])
                zsA = epA.tile([P, ga_ch, 64], f32, tag="zsA")
                for c0 in range(0, ga_ch, CALL_CHUNKS):
                    c1 = min(c0 + CALL_CHUNKS, ga_ch)
                    nc.gpsimd.dma_gather(
                        out_ap=zsA[:, c0:c1, :],
                        in_ap=viewA,
                        idxs_ap=gA_t[:, o16A + c0 * 8:o16A + c1 * 8],
                        num_idxs=(c1 - c0) * P,
                        num_idxs_reg=(c1 - c0) * P,
                        elem_size=64,
                    )
                zsB = epB.tile([P, gb_ch, 64], f32, tag="zsB")
                for c0 in range(0, gb_ch, CALL_CHUNKS):
                    c1 = min(c0 + CALL_CHUNKS, gb_ch)
                    nc.gpsimd.dma_gather(
                        out_ap=zsB[:, c0:c1, :],
                        in_ap=viewB,
                        idxs_ap=gB_t[:, o16B + c0 * 8:o16B + c1 * 8],
                        num_idxs=(c1 - c0) * P,
                        num_idxs_reg=(c1 - c0) * P,
                        elem_size=64,
                    )
                o16A += ga_ch * 8
                o16B += gb_ch * 8
                for b in g:
                    KA, KB = int(KLOs[b]), int(KHIs[b])
                    K = KA + KB
                    ca = int(offA[b] - offA[b0])
                    cb_ = int(offB[b] - offB[b0])
                    adb = adall[:, b, :].unsqueeze(1)
                    et = ep.tile([P, K, 4], f32, tag="et")
                    nc.vector.tensor_add(
                        out=et[:, 0:KA, :],
                        in0=zsA[:, ca:ca + KA, 32:36],
                        in1=adb.broadcast_to([P, KA, 4]))
                    nc.vector.tensor_add(
                        out=et[:, KA:K, :],
                        in0=zsB[:, cb_:cb_ + KB, 32:36],
                        in1=adb.broadcast_to([P, KB, 4]))
                    ex1 = ep.tile([P, K, 4], f32, tag="ex1")
                    nc.scalar.activation(ex1[:], et[:],
                                         mybir.ActivationFunctionType.Exp)
                    ext = ep.tile([P, K, 4], f32, tag="ext")
                    nc.scalar.activation(ext[:], et[:],
                                         mybir.ActivationFunctionType.Exp,
                                         scale=ALPHA)
                    nc.vector.tensor_tensor(out=ext[:], in0=ext[:], in1=ex1[:],
                                            op=mybir.AluOpType.max)
                    # rhs = [ex * zs | ex]
                    rhs_t = ep.tile([P, K, 68], f32, tag="rhs")
                    exb = ext[:].unsqueeze(3)
                    zsbA = zsA[:, ca:ca + KA, 0:32].bitcast(bf16).rearrange(
                        "p k (h f) -> p k h f", h=4)
                    nc.vector.tensor_tensor(
                        out=rhs_t[:, 0:KA, 0:64].rearrange(
                            "p k (h f) -> p k h f", h=4),
                        in0=zsbA,
                        in1=exb[:, 0:KA, :, :].broadcast_to([P, KA, 4, 16]),
                        op=mybir.AluOpType.mult)
                    zsbB = zsB[:, cb_:cb_ + KB, 0:32].bitcast(bf16).rearrange(
                        "p k (h f) -> p k h f", h=4)
                    nc.vector.tensor_tensor(
                        out=rhs_t[:, KA:K, 0:64].rearrange(
                            "p k (h f) -> p k h f", h=4),
                        in0=zsbB,
                        in1=exb[:, KA:K, :, :].broadcast_to([P, KB, 4, 16]),
                        op=mybir.AluOpType.mult)
                    nc.scalar.copy(out=rhs_t[:, :, 64:68], in_=ext[:])
                    # [num | denom]: pairwise tree-sum over chunks (all
                    # operands contiguous, f32 accumulation)
                    n = K
                    while n > 1:
                        hh = n // 2
                        nc.vector.tensor_add(
                            out=rhs_t[:, 0:hh, :],
                            in0=rhs_t[:, 0:hh, :],
                            in1=rhs_t[:, n - hh:n, :])
                        n -= hh
                    red = rhs_t[:, 0, :]
                    rec = fp.tile([P, 4], f32, tag="rec")
                    nc.vector.reciprocal(rec[:], red[:, 64:68])
                    nc.vector.tensor_tensor(
                        out=outS[:, b, :].rearrange("p (h f) -> p h f", h=4),
                        in0=red[:, 0:64].rearrange("p (h f) -> p h f", h=4),
                        in1=rec[:].unsqueeze(2).broadcast_to([P, 4, 16]),
                        op=mybir.AluOpType.mult)

            o3 = outO[:].rearrange("(p s) e -> p s e", s=BLOCKS)
            nc.sync.dma_start(out=o3[:], in_=outS[:])

    nc.finalize()
    return nc


def kernel(h, src, dst, W, A):
    from concourse.bass_utils import run_bass_kernel_spmd

    in_maps, plan, unpack_maps = _build_host_data(h, src, dst, W, A)
    nc = _build_program(plan)
    res = run_bass_kernel_spmd(nc, in_maps, core_ids=list(range(N_CORES)))
    out = np.empty((N_NODES, HF), dtype=np.float32)
    for k in range(N_CORES):
        out[k * NODES_PER_CORE:(k + 1) * NODES_PER_CORE] = \
            res.results[k]["outO"][unpack_maps[k]]
    return out


# revision 19
# speedup vs baseline: 1.2339x; 1.2339x over previous
"""GAT layer (nn_GATLayer) on 8 Trainium2 NeuronCores via Bass/Tile.

Strategy (dst-partitioned; degree-aligned slots, softmax fully local per core):
  - Core k owns dst nodes [k*6250, (k+1)*6250). Each owned node is pinned to a
    (block, partition) slot; ALL of its in-edges occupy that partition across
    the block's chunks. Segment-softmax then needs no scatter at all: the
    per-node sums are free-axis reductions, and the dst attention term ad is a
    per-partition scalar broadcast.
  - Phase A (replicated): zaug[row] = [z bf16 x64 | as f32 x4 | pad] (256B rows)
    for ALL nodes via one matmul with folded weights [W.T | W.T@A1blk].
    Rows are partition-major (row = p*391 + s) so stores are contiguous.
    Two reserved rows (0 and 50047) get as = -60 patched in: pad slots gather
    them and contribute exp(leaky(-60+ad)) ~ 1e-5 to denom and 0 to num (z=0).
  - Phase A0: ad = hT_own @ (W.T@A2blk) for own nodes in (block, pos) order;
    stays SBUF-resident [128, 49, 4].
  - Edge phase per block: dma_gather of zaug rows by src (two overlapping
    int16 views: A = rows [0, 32768), B = rows [17280, 50048); per-node lo/hi
    edge split chosen on host), e = as + ad, ex = max(exp(e), exp(0.2e)),
    rhs = [ex*zs | ex], then one reduce_sum over chunks -> [num | denom];
    out = num * recip(denom), accumulated in SBUF, single store at the end.
  - Gather calls span several blocks (SWDGE ring enlarged to 4096 descs) to
    amortize the per-call desc-gen overhead on Pool.

All index/layout prep (row permutation placing high-out-degree nodes in the
overlapping view region, per-core 2D block packing, per-block KLO/KHI chunk
budgets uniformized across cores so one program serves all 8) is done on host.
"""

import numpy as np

N_NODES = 50000
N_EDGES = 800000
IN_FEATS = 128
OUT_FEATS = 16
NUM_HEADS = 4
ALPHA = 0.2
HF = NUM_HEADS * OUT_FEATS  # 64

N_CORES = 8
P = 128
NODES_PER_CORE = N_NODES // N_CORES     # 6250
BLOCKS = 49                              # ceil(6250/128)
NODE_PAD = BLOCKS * P                    # 6272
NCHUNK = 391                             # zaug chunks; 128*391 = 50048 rows
N_NODES_PAD = P * NCHUNK                 # 50048
VIEW = 32768                             # int16 gather view size
HIB = N_NODES_PAD - VIEW                 # 17280 = base of view B
PAD_A_ROW = 0                            # reserved pad row in view A (p0, s0)
PAD_B_ROW = 96 * NCHUNK                  # reserved pad row in view B (p96, s0)
DMA_SCRATCH = 16384                      # SWDGE ring: 1024 descriptors (HW cap per call)
CALL_CHUNKS = 8                          # max chunks (128 idx each) per gather
TILE_CHUNKS = 40                         # max chunks per stream per zs tile


def _wrap16(vals):
    # gather idx layout: stream position i -> idx tile [16, n/16] at
    # [i%16, i//16]; rows replicated to 128 partitions.
    n = vals.shape[-1]
    w = vals.reshape(n // 16, 16).T                    # [16, n/16]
    return np.tile(w, (8, 1))                          # [128, n/16]


def _plan_groups(klos, khis):
    """Greedy grouping of consecutive blocks into zs-tile groups with
    sum(KLO) <= TILE_CHUNKS and sum(KHI) <= TILE_CHUNKS (single blocks may
    exceed the cap; their gathers are split into CALL_CHUNKS-sized calls)."""
    groups = []
    cur = []
    sa = sb = 0
    for b in range(len(klos)):
        ka, kb = klos[b], khis[b]
        if cur and (sa + ka > TILE_CHUNKS or sb + kb > TILE_CHUNKS):
            groups.append(cur)
            cur, sa, sb = [], 0, 0
        cur.append(b)
        sa += ka
        sb += kb
    if cur:
        groups.append(cur)
    return groups


def _build_host_data(h, src, dst, W, A):
    import ml_dtypes

    src = np.asarray(src)
    dst = np.asarray(dst)
    W = np.asarray(W, dtype=np.float32)
    A = np.asarray(A, dtype=np.float32)
    h = np.asarray(h, dtype=np.float32)

    # folded weights
    A1blk = np.zeros((HF, NUM_HEADS), dtype=np.float32)
    A2blk = np.zeros((HF, NUM_HEADS), dtype=np.float32)
    for hd in range(NUM_HEADS):
        A1blk[hd * OUT_FEATS:(hd + 1) * OUT_FEATS, hd] = A[hd, :OUT_FEATS]
        A2blk[hd * OUT_FEATS:(hd + 1) * OUT_FEATS, hd] = A[hd, OUT_FEATS:]
    WT = np.ascontiguousarray(W.T)                                  # [128, 64]
    wcat = np.concatenate([WT, WT @ A1blk], axis=1).astype(ml_dtypes.bfloat16)
    wad = (WT @ A2blk).astype(ml_dtypes.bfloat16)                   # [128, 4]

    # global row permutation: high-out-degree nodes -> overlap rows
    # [HIB, VIEW); rows 0 and 50047 reserved for pad targets.
    outdeg = np.bincount(src, minlength=N_NODES)
    nodes_by_heat = np.argsort(-outdeg, kind="stable")
    ov_rows = np.arange(HIB, VIEW)
    rest_hi = np.arange(VIEW, N_NODES_PAD)
    rest = np.concatenate(
        [np.arange(1, HIB), rest_hi[rest_hi != PAD_B_ROW]])
    perm = np.empty(N_NODES, dtype=np.int64)
    perm[nodes_by_heat[:len(ov_rows)]] = ov_rows
    perm[nodes_by_heat[len(ov_rows):]] = rest[:N_NODES - len(ov_rows)]

    # hT column for row r: phase A chunk s partition p -> row p*391 + s,
    # processed from hT col s*128 + p.
    hT = np.zeros((P, N_NODES_PAD), dtype=ml_dtypes.bfloat16)
    cols = (perm % NCHUNK) * P + perm // NCHUNK
    hT[:, cols] = h.T.astype(ml_dtypes.bfloat16)

    # per-core edge prep
    order = np.argsort(dst, kind="stable")
    dst_s = dst[order]
    rows_s = perm[src[order]]
    core_begin = np.searchsorted(
        dst_s, np.arange(0, N_NODES + 1, NODES_PER_CORE))

    cores = []
    for k in range(N_CORES):
        lo_e, hi_e = core_begin[k], core_begin[k + 1]
        cd = dst_s[lo_e:hi_e] - k * NODES_PER_CORE
        rw = rows_s[lo_e:hi_e]
        is_a = rw < HIB                      # A-only
        is_b = rw >= VIEW                    # B-only
        is_f = ~is_a & ~is_b                 # flexible
        a = np.bincount(cd[is_a], minlength=NODES_PER_CORE)
        c = np.bincount(cd[is_b], minlength=NODES_PER_CORE)
        f = np.bincount(cd[is_f], minlength=NODES_PER_CORE)
        T = a + c + f
        node_order = np.lexsort((a - c, (-T) // 3))
        # node -> (block, pos)
        node_block = np.empty(NODES_PER_CORE, dtype=np.int64)
        node_pos = np.empty(NODES_PER_CORE, dtype=np.int64)
        node_block[node_order] = np.arange(NODES_PER_CORE) // P
        node_pos[node_order] = np.arange(NODES_PER_CORE) % P
        # per-block requirement maxima (budget components)
        amax = np.zeros(BLOCKS, dtype=np.int64)
        cmax = np.zeros(BLOCKS, dtype=np.int64)
        tmax = np.zeros(BLOCKS, dtype=np.int64)
        for b in range(BLOCKS):
            blk = node_order[b * P:(b + 1) * P]
            amax[b] = a[blk].max()
            cmax[b] = c[blk].max()
            tmax[b] = T[blk].max()
        cores.append(dict(cd=cd, rw=rw, a=a, c=c, f=f, T=T,
                          node_block=node_block, node_pos=node_pos,
                          amax=amax, cmax=cmax, tmax=tmax))

    # minimal uniform per-block chunk budgets across cores: any split with
    # KLO >= max a, KHI >= max c, KLO+KHI >= max T is feasible per node.
    Astar = np.max([co["amax"] for co in cores], axis=0)
    Cstar = np.max([co["cmax"] for co in cores], axis=0)
    Tstar = np.max([co["tmax"] for co in cores], axis=0)
    Kb = np.maximum(Tstar, Astar + Cstar)
    KLOs = np.maximum(Astar, Kb - Cstar)
    KHIs = Kb - KLOs
    groups = _plan_groups(KLOs, KHIs)
    LA = int(KLOs.sum()) * P
    LB = int(KHIs.sum()) * P
    offA = np.concatenate([[0], np.cumsum(KLOs)])    # chunk offsets per block
    offB = np.concatenate([[0], np.cumsum(KHIs)])

    in_maps = []
    unpack_maps = []
    for k in range(N_CORES):
        co = cores[k]
        cd, rw = co["cd"], co["rw"]
        a, f, T = co["a"], co["f"], co["T"]
        node_block, node_pos = co["node_block"], co["node_pos"]
        # per-node lo count: L = max(a, T - KHI_block)
        KHI_n = KHIs[node_block]
        L = np.maximum(a, T - KHI_n)

        # sort edges by (node, flexibility-class) so each node's edge list is
        # [A-only..., flex..., B-only...]; first L edges -> stream A.
        cls = np.where(rw < HIB, 0, np.where(rw < VIEW, 1, 2))
        eo = np.lexsort((cls, cd))
        cd_o, rw_o = cd[eo], rw[eo]
        starts = np.searchsorted(cd_o, np.arange(NODES_PER_CORE + 1))
        rank = np.arange(len(cd_o)) - starts[cd_o]          # rank within node
        to_a = rank < L[cd_o]

        gA = np.full((LA // P, P), PAD_A_ROW, dtype=np.int16)
        gB = np.full((LB // P, P), PAD_B_ROW - HIB, dtype=np.int16)
        # slot chunk = offX[block] + rank (A) or rank - L (B)
        blk_e = node_block[cd_o]
        pos_e = node_pos[cd_o]
        ca = offA[blk_e] + rank
        cb_ = offB[blk_e] + rank - L[cd_o]
        gA[ca[to_a], pos_e[to_a]] = rw_o[to_a].astype(np.int16)
        gB[cb_[~to_a], pos_e[~to_a]] = (rw_o[~to_a] - HIB).astype(np.int16)

        # wrap16 per call group
        gAw, gBw = [], []
        for g in groups:
            b0, b1 = g[0], g[-1] + 1
            gAw.append(_wrap16(gA[offA[b0]:offA[b1]].reshape(-1)))
            gBw.append(_wrap16(gB[offB[b0]:offB[b1]].reshape(-1)))
        gAw = np.ascontiguousarray(np.concatenate(gAw, axis=1))
        gBw = np.ascontiguousarray(np.concatenate(gBw, axis=1))

        # hT_own: col b*128 + pos = h[node]
        hT_own = np.zeros((P, NODE_PAD), dtype=ml_dtypes.bfloat16)
        own = np.arange(k * NODES_PER_CORE, (k + 1) * NODES_PER_CORE)
        hT_own[:, node_block * P + node_pos] = h[own].T.astype(
            ml_dtypes.bfloat16)

        in_maps.append({
            "hT": hT,
            "hT_own": hT_own,
            "wcat": np.ascontiguousarray(wcat),
            "wad": np.ascontiguousarray(wad),
            "gidxA": gAw,
            "gidxB": gBw,
        })
        # outO row for node (block, pos) = pos*BLOCKS + block
        unpack_maps.append(node_pos * BLOCKS + node_block)

    return in_maps, (KLOs, KHIs, groups), unpack_maps


def _build_program(plan):
    import concourse.bacc as bacc
    import concourse.tile as tile
    import concourse.mybir as mybir

    KLOs, KHIs, groups = plan
    LA = int(KLOs.sum()) * P
    LB = int(KHIs.sum()) * P
    f32 = mybir.dt.float32
    bf16 = mybir.dt.bfloat16
    i16 = mybir.dt.int16

    import os as _os
    _simclean = _os.environ.get("SIM_CLEAN", "0") == "1"
    nc = bacc.Bacc("TRN2", target_bir_lowering=False, debug=False,
                   dynamic_dma_scratch_size=DMA_SCRATCH)

    hT = nc.dram_tensor("hT", [P, N_NODES_PAD], bf16, kind="ExternalInput")
    hT_own = nc.dram_tensor("hT_own", [P, NODE_PAD], bf16, kind="ExternalInput")
    wcat_d = nc.dram_tensor("wcat", [P, 68], bf16, kind="ExternalInput")
    wad_d = nc.dram_tensor("wad", [P, 4], bf16, kind="ExternalInput")
    gidxA = nc.dram_tensor("gidxA", [P, LA // 16], i16, kind="ExternalInput")
    gidxB = nc.dram_tensor("gidxB", [P, LB // 16], i16, kind="ExternalInput")

    zaug = nc.dram_tensor("zaug", [N_NODES_PAD, 64], f32)
    outO = nc.dram_tensor("outO", [NODE_PAD, HF], f32, kind="ExternalOutput")

    SC = 4                        # chunks per PSUM tile
    SC2 = 8                       # chunks per load/store superchunk

    with tile.TileContext(nc) as tc:
        with (
            tc.tile_pool(name="const", bufs=1) as cpool,
            tc.tile_pool(name="pa", bufs=8) as pa,
            tc.tile_pool(name="papsum", bufs=4, space="PSUM") as papsum,
            tc.tile_pool(name="adpsum", bufs=1, space="PSUM") as adpsum,
            tc.tile_pool(name="epA", bufs=3) as epA,
            tc.tile_pool(name="epB", bufs=3) as epB,
            tc.tile_pool(name="ep", bufs=3) as ep,
            tc.tile_pool(name="fp", bufs=3) as fp,
        ):
            wcat_t = cpool.tile([P, 68], bf16)
            nc.sync.dma_start(out=wcat_t[:], in_=wcat_d[:])
            wad_t = cpool.tile([P, 4], bf16)
            nc.sync.dma_start(out=wad_t[:], in_=wad_d[:])
            ho_t = cpool.tile([P, NODE_PAD], bf16)
            nc.sync.dma_start(out=ho_t[:], in_=hT_own[:])
            gA_t = cpool.tile([P, LA // 16], i16)
            nc.sync.dma_start(out=gA_t[:], in_=gidxA[:])
            gB_t = cpool.tile([P, LB // 16], i16)
            nc.sync.dma_start(out=gB_t[:], in_=gidxB[:])

            # ---------------- Phase A0: ad for own nodes (SBUF resident) ----
            adp = adpsum.tile([P, BLOCKS, 4], f32)
            for b in range(BLOCKS):
                nc.tensor.matmul(
                    out=adp[:, b, :],
                    lhsT=ho_t[:, b * P:(b + 1) * P],
                    rhs=wad_t[:],
                    start=True, stop=True,
                )
            adall = cpool.tile([P, BLOCKS, 4], f32)
            nc.scalar.copy(out=adall[:], in_=adp[:])

            # ---------------- Phase A: zaug for all nodes -------------------
            n_sc = NCHUNK // SC2
            all_scs = [(s * SC2, SC2) for s in range(n_sc)]
            if NCHUNK % SC2:
                all_scs.append((n_sc * SC2, NCHUNK % SC2))
            z3 = zaug[:].rearrange("(p s) e -> p s e", s=NCHUNK)
            for s0, nsub in all_scs:
                hsl = pa.tile([P, SC2 * P], bf16, tag="hsl")
                nc.sync.dma_start(
                    out=hsl[:, :nsub * P],
                    in_=hT[:, s0 * P:(s0 + nsub) * P])
                zst = pa.tile([P, SC2, 64], f32, tag="zst")
                if _simclean:
                    nc.scalar.memzero(zst[:])
                zbf = zst[:].bitcast(mybir.dt.bfloat16)
                for g0 in range(0, nsub, SC):
                    g1 = min(g0 + SC, nsub)
                    zp = papsum.tile([P, SC, 68], f32, tag="zp")
                    for j in range(g0, g1):
                        nc.tensor.matmul(
                            out=zp[:, j - g0, :],
                            lhsT=hsl[:, j * P:(j + 1) * P],
                            rhs=wcat_t[:],
                            start=True, stop=True,
                        )
                    nc.scalar.copy(out=zbf[:, g0:g1, 0:64],
                                   in_=zp[:, :g1 - g0, 0:64])
                    nc.scalar.copy(out=zst[:, g0:g1, 32:36],
                                   in_=zp[:, :g1 - g0, 64:68])
                # overwrite as = -60 on the reserved pad rows (p, s) =
                # (0, 0) and (96, 0) before the store
                if s0 == 0:
                    nc.vector.memset(zst[0:1, 0, 32:36], -60.0)
                    nc.vector.memset(zst[96:97, 0, 32:36], -60.0)
                nc.sync.dma_start(out=z3[:, s0:s0 + nsub, :],
                                  in_=zst[:, :nsub, :])

            # ---------------- Edge phase ------------------------------------
            viewA = zaug[0:VIEW, :]
            viewB = zaug[HIB:N_NODES_PAD, :]
            outS = cpool.tile([P, BLOCKS, HF], f32)
            offA = np.concatenate([[0], np.cumsum(KLOs)])
            offB = np.concatenate([[0], np.cumsum(KHIs)])
            o16A = o16B = 0
            for g in groups:
                b0, b1 = g[0], g[-1] + 1
                ga_ch = int(offA[b1] - offA[b0])
                gb_ch = int(offB[b1] - offB[b0])
                zsA = epA.tile([P, ga_ch, 64], f32, tag="zsA")
                for c0 in range(0, ga_ch, CALL_CHUNKS):
                    c1 = min(c0 + CALL_CHUNKS, ga_ch)
                    nc.gpsimd.dma_gather(
                        out_ap=zsA[:, c0:c1, :],
                        in_ap=viewA,
                        idxs_ap=gA_t[:, o16A + c0 * 8:o16A + c1 * 8],
                        num_idxs=(c1 - c0) * P,
                        num_idxs_reg=(c1 - c0) * P,
                        elem_size=64,
                    )
                zsB = epB.tile([P, gb_ch, 64], f32, tag="zsB")
                for c0 in range(0, gb_ch, CALL_CHUNKS):
                    c1 = min(c0 + CALL_CHUNKS, gb_ch)
                    nc.gpsimd.dma_gather(
                        out_ap=zsB[:, c0:c1, :],
                        in_ap=viewB,
                        idxs_ap=gB_t[:, o16B + c0 * 8:o16B + c1 * 8],
                        num_idxs=(c1 - c0) * P,
                        num_idxs_reg=(c1 - c0) * P,
                        elem_size=64,
                    )
                o16A += ga_ch * 8
                o16B += gb_ch * 8
                for b in g:
                    KA, KB = int(KLOs[b]), int(KHIs[b])
                    K = KA + KB
                    ca = int(offA[b] - offA[b0])
                    cb_ = int(offB[b] - offB[b0])
                    adb = adall[:, b, :].unsqueeze(1)
                    et = ep.tile([P, K, 4], f32, tag="et")
                    nc.vector.tensor_add(
                        out=et[:, 0:KA, :],
                        in0=zsA[:, ca:ca + KA, 32:36],
                        in1=adb.broadcast_to([P, KA, 4]))
                    nc.vector.tensor_add(
                        out=et[:, KA:K, :],
                        in0=zsB[:, cb_:cb_ + KB, 32:36],
                        in1=adb.broadcast_to([P, KB, 4]))
                    ex1 = ep.tile([P, K, 4], f32, tag="ex1")
                    nc.scalar.activation(ex1[:], et[:],
                                         mybir.ActivationFunctionType.Exp)
                    ext = ep.tile([P, K, 4], f32, tag="ext")
                    nc.scalar.activation(ext[:], et[:],
                                         mybir.ActivationFunctionType.Exp,
                                         scale=ALPHA)
                    nc.vector.tensor_tensor(out=ext[:], in0=ext[:], in1=ex1[:],
                                            op=mybir.AluOpType.max)
                    # rhs = [ex * zs | ex]
                    rhs_t = ep.tile([P, K, 68], f32, tag="rhs")
                    exb = ext[:].unsqueeze(3)
                    zsbA = zsA[:, ca:ca + KA, 0:32].bitcast(bf16).rearrange(
                        "p k (h f) -> p k h f", h=4)
                    nc.vector.tensor_tensor(
                        out=rhs_t[:, 0:KA, 0:64].rearrange(
                            "p k (h f) -> p k h f", h=4),
                        in0=zsbA,
                        in1=exb[:, 0:KA, :, :].broadcast_to([P, KA, 4, 16]),
                        op=mybir.AluOpType.mult)
                    zsbB = zsB[:, cb_:cb_ + KB, 0:32].bitcast(bf16).rearrange(
                        "p k (h f) -> p k h f", h=4)
                    nc.vector.tensor_tensor(
                        out=rhs_t[:, KA:K, 0:64].rearrange(
                            "p k (h f) -> p k h f", h=4),
                        in0=zsbB,
                        in1=exb[:, KA:K, :, :].broadcast_to([P, KB, 4, 16]),
                        op=mybir.AluOpType.mult)
                    nc.scalar.copy(out=rhs_t[:, :, 64:68], in_=ext[:])
                    # [num | denom]: pairwise tree-sum over chunks (all
                    # operands contiguous, f32 accumulation)
                    n = K
                    while n > 1:
                        hh = n // 2
                        nc.vector.tensor_add(
                            out=rhs_t[:, 0:hh, :],
                            in0=rhs_t[:, 0:hh, :],
                            in1=rhs_t[:, n - hh:n, :])
                        n -= hh
                    red = rhs_t[:, 0, :]
                    rec = fp.tile([P, 4], f32, tag="rec")
                    nc.vector.reciprocal(rec[:], red[:, 64:68])
                    nc.vector.tensor_tensor(
                        out=outS[:, b, :].rearrange("p (h f) -> p h f", h=4),
                        in0=red[:, 0:64].rearrange("p (h f) -> p h f", h=4),
                        in1=rec[:].unsqueeze(2).broadcast_to([P, 4, 16]),
                        op=mybir.AluOpType.mult)

            o3 = outO[:].rearrange("(p s) e -> p s e", s=BLOCKS)
            nc.sync.dma_start(out=o3[:], in_=outS[:])

    nc.finalize()
    return nc


def kernel(h, src, dst, W, A):
    from concourse.bass_utils import run_bass_kernel_spmd

    in_maps, plan, unpack_maps = _build_host_data(h, src, dst, W, A)
    nc = _build_program(plan)
    res = run_bass_kernel_spmd(nc, in_maps, core_ids=list(range(N_CORES)))
    out = np.empty((N_NODES, HF), dtype=np.float32)
    for k in range(N_CORES):
        out[k * NODES_PER_CORE:(k + 1) * NODES_PER_CORE] = \
            res.results[k]["outO"][unpack_maps[k]]
    return out


# revision 21
# speedup vs baseline: 1.3037x; 1.0565x over previous
"""GAT layer (nn_GATLayer) on 8 Trainium2 NeuronCores via Bass/Tile.

Strategy (dst-partitioned; degree-aligned slots, softmax fully local per core):
  - Core k owns dst nodes [k*6250, (k+1)*6250). Each owned node is pinned to a
    (block, partition) slot; ALL of its in-edges occupy that partition across
    the block's chunks. Segment-softmax then needs no scatter at all: the
    per-node sums are free-axis reductions, and the dst attention term ad is a
    per-partition scalar broadcast.
  - Phase A (replicated): zaug[row] = [z bf16 x64 | as f32 x4 | pad] (256B rows)
    for ALL nodes via one matmul with folded weights [W.T | W.T@A1blk].
    Rows are partition-major (row = p*391 + s) so stores are contiguous.
    Two reserved rows (0 and 50047) get as = -60 patched in: pad slots gather
    them and contribute exp(leaky(-60+ad)) ~ 1e-5 to denom and 0 to num (z=0).
  - Phase A0: ad = hT_own @ (W.T@A2blk) for own nodes in (block, pos) order;
    stays SBUF-resident [128, 49, 4].
  - Edge phase per block: dma_gather of zaug rows by src (two overlapping
    int16 views: A = rows [0, 32768), B = rows [17280, 50048); per-node lo/hi
    edge split chosen on host), e = as + ad, ex = max(exp(e), exp(0.2e)),
    rhs = [ex*zs | ex], then one reduce_sum over chunks -> [num | denom];
    out = num * recip(denom), accumulated in SBUF, single store at the end.
  - Gather calls span several blocks (SWDGE ring enlarged to 4096 descs) to
    amortize the per-call desc-gen overhead on Pool.

All index/layout prep (row permutation placing high-out-degree nodes in the
overlapping view region, per-core 2D block packing, per-block KLO/KHI chunk
budgets uniformized across cores so one program serves all 8) is done on host.
"""

import numpy as np

N_NODES = 50000
N_EDGES = 800000
IN_FEATS = 128
OUT_FEATS = 16
NUM_HEADS = 4
ALPHA = 0.2
HF = NUM_HEADS * OUT_FEATS  # 64

N_CORES = 8
P = 128
NODES_PER_CORE = N_NODES // N_CORES     # 6250
BLOCKS = 49                              # ceil(6250/128)
NODE_PAD = BLOCKS * P                    # 6272
NCHUNK = 391                             # zaug chunks; 128*391 = 50048 rows
N_NODES_PAD = P * NCHUNK                 # 50048
VIEW = 32768                             # int16 gather view size
HIB = N_NODES_PAD - VIEW                 # 17280 = base of view B
PAD_A_ROW = 0                            # reserved pad row in view A (p0, s0)
PAD_B_ROW = 96 * NCHUNK                  # reserved pad row in view B (p96, s0)
DMA_SCRATCH = 16384                      # SWDGE ring: 1024 descriptors (HW cap per call)
CALL_CHUNKS = 8                          # max chunks (128 idx each) per gather
TILE_CHUNKS = 40                         # max chunks per stream per zs tile


def _wrap16(vals):
    # gather idx layout: stream position i -> idx tile [16, n/16] at
    # [i%16, i//16]; rows replicated to 128 partitions.
    n = vals.shape[-1]
    w = vals.reshape(n // 16, 16).T                    # [16, n/16]
    return np.tile(w, (8, 1))                          # [128, n/16]


def _plan_groups(klos, khis):
    """Greedy grouping of consecutive blocks into zs-tile groups with
    sum(KLO) <= TILE_CHUNKS and sum(KHI) <= TILE_CHUNKS (single blocks may
    exceed the cap; their gathers are split into CALL_CHUNKS-sized calls)."""
    groups = []
    cur = []
    sa = sb = 0
    for b in range(len(klos)):
        ka, kb = klos[b], khis[b]
        if cur and (sa + ka > TILE_CHUNKS or sb + kb > TILE_CHUNKS):
            groups.append(cur)
            cur, sa, sb = [], 0, 0
        cur.append(b)
        sa += ka
        sb += kb
    if cur:
        groups.append(cur)
    return groups


def _build_host_data(h, src, dst, W, A):
    import ml_dtypes

    src = np.asarray(src)
    dst = np.asarray(dst)
    W = np.asarray(W, dtype=np.float32)
    A = np.asarray(A, dtype=np.float32)
    h = np.asarray(h, dtype=np.float32)

    # folded weights
    A1blk = np.zeros((HF, NUM_HEADS), dtype=np.float32)
    A2blk = np.zeros((HF, NUM_HEADS), dtype=np.float32)
    for hd in range(NUM_HEADS):
        A1blk[hd * OUT_FEATS:(hd + 1) * OUT_FEATS, hd] = A[hd, :OUT_FEATS]
        A2blk[hd * OUT_FEATS:(hd + 1) * OUT_FEATS, hd] = A[hd, OUT_FEATS:]
    WT = np.ascontiguousarray(W.T)                                  # [128, 64]
    wcat = np.concatenate([WT, WT @ A1blk], axis=1).astype(ml_dtypes.bfloat16)
    wad = (WT @ A2blk).astype(ml_dtypes.bfloat16)                   # [128, 4]

    # global row permutation: high-out-degree nodes -> overlap rows
    # [HIB, VIEW); rows 0 and 50047 reserved for pad targets.
    outdeg = np.bincount(src, minlength=N_NODES)
    nodes_by_heat = np.argsort(-outdeg, kind="stable")
    ov_rows = np.arange(HIB, VIEW)
    rest_hi = np.arange(VIEW, N_NODES_PAD)
    rest = np.concatenate(
        [np.arange(1, HIB), rest_hi[rest_hi != PAD_B_ROW]])
    perm = np.empty(N_NODES, dtype=np.int64)
    perm[nodes_by_heat[:len(ov_rows)]] = ov_rows
    perm[nodes_by_heat[len(ov_rows):]] = rest[:N_NODES - len(ov_rows)]

    # hT column for row r: phase A chunk s partition p -> row p*391 + s,
    # processed from hT col s*128 + p.
    hT = np.zeros((P, N_NODES_PAD), dtype=ml_dtypes.bfloat16)
    cols = (perm % NCHUNK) * P + perm // NCHUNK
    hT[:, cols] = h.T.astype(ml_dtypes.bfloat16)

    # per-core edge prep
    order = np.argsort(dst, kind="stable")
    dst_s = dst[order]
    rows_s = perm[src[order]]
    core_begin = np.searchsorted(
        dst_s, np.arange(0, N_NODES + 1, NODES_PER_CORE))

    cores = []
    for k in range(N_CORES):
        lo_e, hi_e = core_begin[k], core_begin[k + 1]
        cd = dst_s[lo_e:hi_e] - k * NODES_PER_CORE
        rw = rows_s[lo_e:hi_e]
        is_a = rw < HIB                      # A-only
        is_b = rw >= VIEW                    # B-only
        is_f = ~is_a & ~is_b                 # flexible
        a = np.bincount(cd[is_a], minlength=NODES_PER_CORE)
        c = np.bincount(cd[is_b], minlength=NODES_PER_CORE)
        f = np.bincount(cd[is_f], minlength=NODES_PER_CORE)
        T = a + c + f
        node_order = np.lexsort((a - c, (-T) // 3))
        # node -> (block, pos)
        node_block = np.empty(NODES_PER_CORE, dtype=np.int64)
        node_pos = np.empty(NODES_PER_CORE, dtype=np.int64)
        node_block[node_order] = np.arange(NODES_PER_CORE) // P
        node_pos[node_order] = np.arange(NODES_PER_CORE) % P
        # per-block requirement maxima (budget components)
        amax = np.zeros(BLOCKS, dtype=np.int64)
        cmax = np.zeros(BLOCKS, dtype=np.int64)
        tmax = np.zeros(BLOCKS, dtype=np.int64)
        for b in range(BLOCKS):
            blk = node_order[b * P:(b + 1) * P]
            amax[b] = a[blk].max()
            cmax[b] = c[blk].max()
            tmax[b] = T[blk].max()
        cores.append(dict(cd=cd, rw=rw, a=a, c=c, f=f, T=T,
                          node_block=node_block, node_pos=node_pos,
                          amax=amax, cmax=cmax, tmax=tmax))

    # minimal uniform per-block chunk budgets across cores: any split with
    # KLO >= max a, KHI >= max c, KLO+KHI >= max T is feasible per node.
    Astar = np.max([co["amax"] for co in cores], axis=0)
    Cstar = np.max([co["cmax"] for co in cores], axis=0)
    Tstar = np.max([co["tmax"] for co in cores], axis=0)
    Kb = np.maximum(Tstar, Astar + Cstar)
    KLOs = np.maximum(Astar, Kb - Cstar)
    KHIs = Kb - KLOs
    groups = _plan_groups(KLOs, KHIs)
    LA = int(KLOs.sum()) * P
    LB = int(KHIs.sum()) * P
    offA = np.concatenate([[0], np.cumsum(KLOs)])    # chunk offsets per block
    offB = np.concatenate([[0], np.cumsum(KHIs)])

    in_maps = []
    unpack_maps = []
    for k in range(N_CORES):
        co = cores[k]
        cd, rw = co["cd"], co["rw"]
        a, f, T = co["a"], co["f"], co["T"]
        node_block, node_pos = co["node_block"], co["node_pos"]
        # per-node lo count: L = max(a, T - KHI_block)
        KHI_n = KHIs[node_block]
        L = np.maximum(a, T - KHI_n)

        # sort edges by (node, flexibility-class) so each node's edge list is
        # [A-only..., flex..., B-only...]; first L edges -> stream A.
        cls = np.where(rw < HIB, 0, np.where(rw < VIEW, 1, 2))
        eo = np.lexsort((cls, cd))
        cd_o, rw_o = cd[eo], rw[eo]
        starts = np.searchsorted(cd_o, np.arange(NODES_PER_CORE + 1))
        rank = np.arange(len(cd_o)) - starts[cd_o]          # rank within node
        to_a = rank < L[cd_o]

        gA = np.full((LA // P, P), PAD_A_ROW, dtype=np.int16)
        gB = np.full((LB // P, P), PAD_B_ROW - HIB, dtype=np.int16)
        # slot chunk = offX[block] + rank (A) or rank - L (B)
        blk_e = node_block[cd_o]
        pos_e = node_pos[cd_o]
        ca = offA[blk_e] + rank
        cb_ = offB[blk_e] + rank - L[cd_o]
        gA[ca[to_a], pos_e[to_a]] = rw_o[to_a].astype(np.int16)
        gB[cb_[~to_a], pos_e[~to_a]] = (rw_o[~to_a] - HIB).astype(np.int16)

        # wrap16 per call group
        gAw, gBw = [], []
        for g in groups:
            b0, b1 = g[0], g[-1] + 1
            gAw.append(_wrap16(gA[offA[b0]:offA[b1]].reshape(-1)))
            gBw.append(_wrap16(gB[offB[b0]:offB[b1]].reshape(-1)))
        gAw = np.ascontiguousarray(np.concatenate(gAw, axis=1))
        gBw = np.ascontiguousarray(np.concatenate(gBw, axis=1))

        # hT_own: col b*128 + pos = h[node]
        hT_own = np.zeros((P, NODE_PAD), dtype=ml_dtypes.bfloat16)
        own = np.arange(k * NODES_PER_CORE, (k + 1) * NODES_PER_CORE)
        hT_own[:, node_block * P + node_pos] = h[own].T.astype(
            ml_dtypes.bfloat16)

        in_maps.append({
            "hT": hT,
            "hT_own": hT_own,
            "wcat": np.ascontiguousarray(wcat),
            "wad": np.ascontiguousarray(wad),
            "gidxA": gAw,
            "gidxB": gBw,
        })
        # outO row for node (block, pos) = pos*BLOCKS + block
        unpack_maps.append(node_pos * BLOCKS + node_block)

    return in_maps, (KLOs, KHIs, groups), unpack_maps


def _build_program(plan):
    import concourse.bacc as bacc
    import concourse.tile as tile
    import concourse.mybir as mybir

    KLOs, KHIs, groups = plan
    LA = int(KLOs.sum()) * P
    LB = int(KHIs.sum()) * P
    f32 = mybir.dt.float32
    bf16 = mybir.dt.bfloat16
    i16 = mybir.dt.int16

    import os as _os
    _simclean = _os.environ.get("SIM_CLEAN", "0") == "1"
    nc = bacc.Bacc("TRN2", target_bir_lowering=False, debug=False,
                   dynamic_dma_scratch_size=DMA_SCRATCH)

    hT = nc.dram_tensor("hT", [P, N_NODES_PAD], bf16, kind="ExternalInput")
    hT_own = nc.dram_tensor("hT_own", [P, NODE_PAD], bf16, kind="ExternalInput")
    wcat_d = nc.dram_tensor("wcat", [P, 68], bf16, kind="ExternalInput")
    wad_d = nc.dram_tensor("wad", [P, 4], bf16, kind="ExternalInput")
    gidxA = nc.dram_tensor("gidxA", [P, LA // 16], i16, kind="ExternalInput")
    gidxB = nc.dram_tensor("gidxB", [P, LB // 16], i16, kind="ExternalInput")

    zaug = nc.dram_tensor("zaug", [N_NODES_PAD, 64], f32)
    outO = nc.dram_tensor("outO", [NODE_PAD, HF], f32, kind="ExternalOutput")

    SC = 4                        # chunks per PSUM tile
    SC2 = 8                       # chunks per load/store superchunk

    with tile.TileContext(nc) as tc:
        with (
            tc.tile_pool(name="const", bufs=1) as cpool,
            tc.tile_pool(name="pa", bufs=8) as pa,
            tc.tile_pool(name="papsum", bufs=4, space="PSUM") as papsum,
            tc.tile_pool(name="adpsum", bufs=1, space="PSUM") as adpsum,
            tc.tile_pool(name="epA", bufs=3) as epA,
            tc.tile_pool(name="epB", bufs=3) as epB,
            tc.tile_pool(name="ep", bufs=4) as ep,
            tc.tile_pool(name="fp", bufs=3) as fp,
        ):
            wcat_t = cpool.tile([P, 68], bf16)
            nc.sync.dma_start(out=wcat_t[:], in_=wcat_d[:])
            wad_t = cpool.tile([P, 4], bf16)
            nc.sync.dma_start(out=wad_t[:], in_=wad_d[:])
            ho_t = cpool.tile([P, NODE_PAD], bf16)
            nc.sync.dma_start(out=ho_t[:], in_=hT_own[:])
            gA_t = cpool.tile([P, LA // 16], i16)
            nc.sync.dma_start(out=gA_t[:], in_=gidxA[:])
            gB_t = cpool.tile([P, LB // 16], i16)
            nc.sync.dma_start(out=gB_t[:], in_=gidxB[:])

            # ---------------- Phase A0: ad for own nodes (SBUF resident) ----
            adp = adpsum.tile([P, BLOCKS, 4], f32)
            for b in range(BLOCKS):
                nc.tensor.matmul(
                    out=adp[:, b, :],
                    lhsT=ho_t[:, b * P:(b + 1) * P],
                    rhs=wad_t[:],
                    start=True, stop=True,
                )
            adall = cpool.tile([P, BLOCKS, 4], f32)
            nc.scalar.copy(out=adall[:], in_=adp[:])

            # ---------------- Phase A: zaug for all nodes -------------------
            n_sc = NCHUNK // SC2
            all_scs = [(s * SC2, SC2) for s in range(n_sc)]
            if NCHUNK % SC2:
                all_scs.append((n_sc * SC2, NCHUNK % SC2))
            z3 = zaug[:].rearrange("(p s) e -> p s e", s=NCHUNK)
            for s0, nsub in all_scs:
                hsl = pa.tile([P, SC2 * P], bf16, tag="hsl")
                nc.sync.dma_start(
                    out=hsl[:, :nsub * P],
                    in_=hT[:, s0 * P:(s0 + nsub) * P])
                zst = pa.tile([P, SC2, 64], f32, tag="zst")
                if _simclean:
                    nc.scalar.memzero(zst[:])
                zbf = zst[:].bitcast(mybir.dt.bfloat16)
                for g0 in range(0, nsub, SC):
                    g1 = min(g0 + SC, nsub)
                    zp = papsum.tile([P, SC, 68], f32, tag="zp")
                    for j in range(g0, g1):
                        nc.tensor.matmul(
                            out=zp[:, j - g0, :],
                            lhsT=hsl[:, j * P:(j + 1) * P],
                            rhs=wcat_t[:],
                            start=True, stop=True,
                        )
                    nc.scalar.copy(out=zbf[:, g0:g1, 0:64],
                                   in_=zp[:, :g1 - g0, 0:64])
                    nc.scalar.copy(out=zst[:, g0:g1, 32:36],
                                   in_=zp[:, :g1 - g0, 64:68])
                # overwrite as = -60 on the reserved pad rows (p, s) =
                # (0, 0) and (96, 0) before the store
                if s0 == 0:
                    nc.vector.memset(zst[0:1, 0, 32:36], -60.0)
                    nc.vector.memset(zst[96:97, 0, 32:36], -60.0)
                nc.sync.dma_start(out=z3[:, s0:s0 + nsub, :],
                                  in_=zst[:, :nsub, :])

            # ---------------- Edge phase ------------------------------------
            viewA = zaug[0:VIEW, :]
            viewB = zaug[HIB:N_NODES_PAD, :]
            outS = cpool.tile([P, BLOCKS, HF], f32)
            offA = np.concatenate([[0], np.cumsum(KLOs)])
            offB = np.concatenate([[0], np.cumsum(KHIs)])
            o16A = o16B = 0
            for g in groups:
                b0, b1 = g[0], g[-1] + 1
                ga_ch = int(offA[b1] - offA[b0])
                gb_ch = int(offB[b1] - offB[b0])
                zsA = epA.tile([P, ga_ch, 64], f32, tag="zsA")
                for c0 in range(0, ga_ch, CALL_CHUNKS):
                    c1 = min(c0 + CALL_CHUNKS, ga_ch)
                    nc.gpsimd.dma_gather(
                        out_ap=zsA[:, c0:c1, :],
                        in_ap=viewA,
                        idxs_ap=gA_t[:, o16A + c0 * 8:o16A + c1 * 8],
                        num_idxs=(c1 - c0) * P,
                        num_idxs_reg=(c1 - c0) * P,
                        elem_size=64,
                    )
                zsB = epB.tile([P, gb_ch, 64], f32, tag="zsB")
                for c0 in range(0, gb_ch, CALL_CHUNKS):
                    c1 = min(c0 + CALL_CHUNKS, gb_ch)
                    nc.gpsimd.dma_gather(
                        out_ap=zsB[:, c0:c1, :],
                        in_ap=viewB,
                        idxs_ap=gB_t[:, o16B + c0 * 8:o16B + c1 * 8],
                        num_idxs=(c1 - c0) * P,
                        num_idxs_reg=(c1 - c0) * P,
                        elem_size=64,
                    )
                o16A += ga_ch * 8
                o16B += gb_ch * 8
                for b in g:
                    KA, KB = int(KLOs[b]), int(KHIs[b])
                    K = KA + KB
                    ca = int(offA[b] - offA[b0])
                    cb_ = int(offB[b] - offB[b0])
                    adb = adall[:, b, :].unsqueeze(1)
                    et = ep.tile([P, K, 4], f32, tag="et")
                    nc.vector.tensor_add(
                        out=et[:, 0:KA, :],
                        in0=zsA[:, ca:ca + KA, 32:36],
                        in1=adb.broadcast_to([P, KA, 4]))
                    nc.vector.tensor_add(
                        out=et[:, KA:K, :],
                        in0=zsB[:, cb_:cb_ + KB, 32:36],
                        in1=adb.broadcast_to([P, KB, 4]))
                    ex1 = ep.tile([P, K, 4], bf16, tag="ex1")
                    nc.scalar.activation(ex1[:], et[:],
                                         mybir.ActivationFunctionType.Exp)
                    ext = ep.tile([P, K, 4], bf16, tag="ext")
                    nc.scalar.activation(ext[:], et[:],
                                         mybir.ActivationFunctionType.Exp,
                                         scale=ALPHA)
                    nc.vector.tensor_tensor(out=ext[:], in0=ext[:], in1=ex1[:],
                                            op=mybir.AluOpType.max)
                    # rhs = [ex * zs | ex]
                    rhs_t = ep.tile([P, K, 68], bf16, tag="rhs")
                    exb = ext[:].unsqueeze(3)
                    zsbA = zsA[:, ca:ca + KA, 0:32].bitcast(bf16).rearrange(
                        "p k (h f) -> p k h f", h=4)
                    nc.vector.tensor_tensor(
                        out=rhs_t[:, 0:KA, 0:64].rearrange(
                            "p k (h f) -> p k h f", h=4),
                        in0=zsbA,
                        in1=exb[:, 0:KA, :, :].broadcast_to([P, KA, 4, 16]),
                        op=mybir.AluOpType.mult)
                    zsbB = zsB[:, cb_:cb_ + KB, 0:32].bitcast(bf16).rearrange(
                        "p k (h f) -> p k h f", h=4)
                    nc.vector.tensor_tensor(
                        out=rhs_t[:, KA:K, 0:64].rearrange(
                            "p k (h f) -> p k h f", h=4),
                        in0=zsbB,
                        in1=exb[:, KA:K, :, :].broadcast_to([P, KB, 4, 16]),
                        op=mybir.AluOpType.mult)
                    nc.scalar.copy(out=rhs_t[:, :, 64:68], in_=ext[:])
                    # [num | denom]: pairwise tree-sum over chunks (all
                    # operands contiguous, f32 accumulation)
                    n = K
                    while n > 1:
                        hh = n // 2
                        nc.vector.tensor_add(
                            out=rhs_t[:, 0:hh, :],
                            in0=rhs_t[:, 0:hh, :],
                            in1=rhs_t[:, n - hh:n, :])
                        n -= hh
                    red = rhs_t[:, 0, :]
                    rec = fp.tile([P, 4], f32, tag="rec")
                    nc.vector.reciprocal(rec[:], red[:, 64:68])
                    nc.vector.tensor_tensor(
                        out=outS[:, b, :].rearrange("p (h f) -> p h f", h=4),
                        in0=red[:, 0:64].rearrange("p (h f) -> p h f", h=4),
                        in1=rec[:].unsqueeze(2).broadcast_to([P, 4, 16]),
                        op=mybir.AluOpType.mult)

            o3 = outO[:].rearrange("(p s) e -> p s e", s=BLOCKS)
            nc.sync.dma_start(out=o3[:], in_=outS[:])

    nc.finalize()
    return nc


def kernel(h, src, dst, W, A):
    from concourse.bass_utils import run_bass_kernel_spmd

    in_maps, plan, unpack_maps = _build_host_data(h, src, dst, W, A)
    nc = _build_program(plan)
    res = run_bass_kernel_spmd(nc, in_maps, core_ids=list(range(N_CORES)))
    out = np.empty((N_NODES, HF), dtype=np.float32)
    for k in range(N_CORES):
        out[k * NODES_PER_CORE:(k + 1) * NODES_PER_CORE] = \
            res.results[k]["outO"][unpack_maps[k]]
    return out


# revision 22
# speedup vs baseline: 1.3113x; 1.0058x over previous
"""GAT layer (nn_GATLayer) on 8 Trainium2 NeuronCores via Bass/Tile.

Strategy (dst-partitioned; degree-aligned slots, softmax fully local per core):
  - Core k owns dst nodes [k*6250, (k+1)*6250). Each owned node is pinned to a
    (block, partition) slot; ALL of its in-edges occupy that partition across
    the block's chunks. Segment-softmax then needs no scatter at all: the
    per-node sums are free-axis reductions, and the dst attention term ad is a
    per-partition scalar broadcast.
  - Phase A (replicated): zaug[row] = [z bf16 x64 | as f32 x4 | pad] (256B rows)
    for ALL nodes via one matmul with folded weights [W.T | W.T@A1blk].
    Rows are partition-major (row = p*391 + s) so stores are contiguous.
    Two reserved rows (0 and 50047) get as = -60 patched in: pad slots gather
    them and contribute exp(leaky(-60+ad)) ~ 1e-5 to denom and 0 to num (z=0).
  - Phase A0: ad = hT_own @ (W.T@A2blk) for own nodes in (block, pos) order;
    stays SBUF-resident [128, 49, 4].
  - Edge phase per block: dma_gather of zaug rows by src (two overlapping
    int16 views: A = rows [0, 32768), B = rows [17280, 50048); per-node lo/hi
    edge split chosen on host), e = as + ad, ex = max(exp(e), exp(0.2e)),
    rhs = [ex*zs | ex], then one reduce_sum over chunks -> [num | denom];
    out = num * recip(denom), accumulated in SBUF, single store at the end.
  - Gather calls span several blocks (SWDGE ring enlarged to 4096 descs) to
    amortize the per-call desc-gen overhead on Pool.

All index/layout prep (row permutation placing high-out-degree nodes in the
overlapping view region, per-core 2D block packing, per-block KLO/KHI chunk
budgets uniformized across cores so one program serves all 8) is done on host.
"""

import numpy as np

N_NODES = 50000
N_EDGES = 800000
IN_FEATS = 128
OUT_FEATS = 16
NUM_HEADS = 4
ALPHA = 0.2
HF = NUM_HEADS * OUT_FEATS  # 64

N_CORES = 8
P = 128
NODES_PER_CORE = N_NODES // N_CORES     # 6250
BLOCKS = 49                              # ceil(6250/128)
NODE_PAD = BLOCKS * P                    # 6272
NCHUNK = 391                             # zaug chunks; 128*391 = 50048 rows
N_NODES_PAD = P * NCHUNK                 # 50048
VIEW = 32768                             # int16 gather view size
HIB = N_NODES_PAD - VIEW                 # 17280 = base of view B
PAD_A_ROW = 0                            # reserved pad row in view A (p0, s0)
PAD_B_ROW = 96 * NCHUNK                  # reserved pad row in view B (p96, s0)
DMA_SCRATCH = 16384                      # SWDGE ring: 1024 descriptors (HW cap per call)
CALL_CHUNKS = 8                          # max chunks (128 idx each) per gather
TILE_CHUNKS = 40                         # max chunks per stream per zs tile
GE = 72                                  # gather elem: 72 bf16 = 144B of each 256B row


def _wrap16(vals):
    # gather idx layout: stream position i -> idx tile [16, n/16] at
    # [i%16, i//16]; rows replicated to 128 partitions.
    n = vals.shape[-1]
    w = vals.reshape(n // 16, 16).T                    # [16, n/16]
    return np.tile(w, (8, 1))                          # [128, n/16]


def _plan_groups(klos, khis):
    """Greedy grouping of consecutive blocks into zs-tile groups with
    sum(KLO) <= TILE_CHUNKS and sum(KHI) <= TILE_CHUNKS (single blocks may
    exceed the cap; their gathers are split into CALL_CHUNKS-sized calls)."""
    groups = []
    cur = []
    sa = sb = 0
    for b in range(len(klos)):
        ka, kb = klos[b], khis[b]
        if cur and (sa + ka > TILE_CHUNKS or sb + kb > TILE_CHUNKS):
            groups.append(cur)
            cur, sa, sb = [], 0, 0
        cur.append(b)
        sa += ka
        sb += kb
    if cur:
        groups.append(cur)
    return groups


def _build_host_data(h, src, dst, W, A):
    import ml_dtypes

    src = np.asarray(src)
    dst = np.asarray(dst)
    W = np.asarray(W, dtype=np.float32)
    A = np.asarray(A, dtype=np.float32)
    h = np.asarray(h, dtype=np.float32)

    # folded weights
    A1blk = np.zeros((HF, NUM_HEADS), dtype=np.float32)
    A2blk = np.zeros((HF, NUM_HEADS), dtype=np.float32)
    for hd in range(NUM_HEADS):
        A1blk[hd * OUT_FEATS:(hd + 1) * OUT_FEATS, hd] = A[hd, :OUT_FEATS]
        A2blk[hd * OUT_FEATS:(hd + 1) * OUT_FEATS, hd] = A[hd, OUT_FEATS:]
    WT = np.ascontiguousarray(W.T)                                  # [128, 64]
    wcat = np.concatenate([WT, WT @ A1blk], axis=1).astype(ml_dtypes.bfloat16)
    wad = (WT @ A2blk).astype(ml_dtypes.bfloat16)                   # [128, 4]

    # global row permutation: high-out-degree nodes -> overlap rows
    # [HIB, VIEW); rows 0 and 50047 reserved for pad targets.
    outdeg = np.bincount(src, minlength=N_NODES)
    nodes_by_heat = np.argsort(-outdeg, kind="stable")
    ov_rows = np.arange(HIB, VIEW)
    rest_hi = np.arange(VIEW, N_NODES_PAD)
    rest = np.concatenate(
        [np.arange(1, HIB), rest_hi[rest_hi != PAD_B_ROW]])
    perm = np.empty(N_NODES, dtype=np.int64)
    perm[nodes_by_heat[:len(ov_rows)]] = ov_rows
    perm[nodes_by_heat[len(ov_rows):]] = rest[:N_NODES - len(ov_rows)]

    # hT column for row r: phase A chunk s partition p -> row p*391 + s,
    # processed from hT col s*128 + p.
    hT = np.zeros((P, N_NODES_PAD), dtype=ml_dtypes.bfloat16)
    cols = (perm % NCHUNK) * P + perm // NCHUNK
    hT[:, cols] = h.T.astype(ml_dtypes.bfloat16)

    # per-core edge prep
    order = np.argsort(dst, kind="stable")
    dst_s = dst[order]
    rows_s = perm[src[order]]
    core_begin = np.searchsorted(
        dst_s, np.arange(0, N_NODES + 1, NODES_PER_CORE))

    cores = []
    for k in range(N_CORES):
        lo_e, hi_e = core_begin[k], core_begin[k + 1]
        cd = dst_s[lo_e:hi_e] - k * NODES_PER_CORE
        rw = rows_s[lo_e:hi_e]
        is_a = rw < HIB                      # A-only
        is_b = rw >= VIEW                    # B-only
        is_f = ~is_a & ~is_b                 # flexible
        a = np.bincount(cd[is_a], minlength=NODES_PER_CORE)
        c = np.bincount(cd[is_b], minlength=NODES_PER_CORE)
        f = np.bincount(cd[is_f], minlength=NODES_PER_CORE)
        T = a + c + f
        node_order = np.lexsort((a - c, (-T) // 3))
        # node -> (block, pos)
        node_block = np.empty(NODES_PER_CORE, dtype=np.int64)
        node_pos = np.empty(NODES_PER_CORE, dtype=np.int64)
        node_block[node_order] = np.arange(NODES_PER_CORE) // P
        node_pos[node_order] = np.arange(NODES_PER_CORE) % P
        # per-block requirement maxima (budget components)
        amax = np.zeros(BLOCKS, dtype=np.int64)
        cmax = np.zeros(BLOCKS, dtype=np.int64)
        tmax = np.zeros(BLOCKS, dtype=np.int64)
        for b in range(BLOCKS):
            blk = node_order[b * P:(b + 1) * P]
            amax[b] = a[blk].max()
            cmax[b] = c[blk].max()
            tmax[b] = T[blk].max()
        cores.append(dict(cd=cd, rw=rw, a=a, c=c, f=f, T=T,
                          node_block=node_block, node_pos=node_pos,
                          amax=amax, cmax=cmax, tmax=tmax))

    # minimal uniform per-block chunk budgets across cores: any split with
    # KLO >= max a, KHI >= max c, KLO+KHI >= max T is feasible per node.
    Astar = np.max([co["amax"] for co in cores], axis=0)
    Cstar = np.max([co["cmax"] for co in cores], axis=0)
    Tstar = np.max([co["tmax"] for co in cores], axis=0)
    Kb = np.maximum(Tstar, Astar + Cstar)
    KLOs = np.maximum(Astar, Kb - Cstar)
    KHIs = Kb - KLOs
    groups = _plan_groups(KLOs, KHIs)
    LA = int(KLOs.sum()) * P
    LB = int(KHIs.sum()) * P
    offA = np.concatenate([[0], np.cumsum(KLOs)])    # chunk offsets per block
    offB = np.concatenate([[0], np.cumsum(KHIs)])

    in_maps = []
    unpack_maps = []
    for k in range(N_CORES):
        co = cores[k]
        cd, rw = co["cd"], co["rw"]
        a, f, T = co["a"], co["f"], co["T"]
        node_block, node_pos = co["node_block"], co["node_pos"]
        # per-node lo count: L = max(a, T - KHI_block)
        KHI_n = KHIs[node_block]
        L = np.maximum(a, T - KHI_n)

        # sort edges by (node, flexibility-class) so each node's edge list is
        # [A-only..., flex..., B-only...]; first L edges -> stream A.
        cls = np.where(rw < HIB, 0, np.where(rw < VIEW, 1, 2))
        eo = np.lexsort((cls, cd))
        cd_o, rw_o = cd[eo], rw[eo]
        starts = np.searchsorted(cd_o, np.arange(NODES_PER_CORE + 1))
        rank = np.arange(len(cd_o)) - starts[cd_o]          # rank within node
        to_a = rank < L[cd_o]

        gA = np.full((LA // P, P), PAD_A_ROW, dtype=np.int16)
        gB = np.full((LB // P, P), PAD_B_ROW - HIB, dtype=np.int16)
        # slot chunk = offX[block] + rank (A) or rank - L (B)
        blk_e = node_block[cd_o]
        pos_e = node_pos[cd_o]
        ca = offA[blk_e] + rank
        cb_ = offB[blk_e] + rank - L[cd_o]
        gA[ca[to_a], pos_e[to_a]] = rw_o[to_a].astype(np.int16)
        gB[cb_[~to_a], pos_e[~to_a]] = (rw_o[~to_a] - HIB).astype(np.int16)

        # wrap16 per call group
        gAw, gBw = [], []
        for g in groups:
            b0, b1 = g[0], g[-1] + 1
            gAw.append(_wrap16(gA[offA[b0]:offA[b1]].reshape(-1)))
            gBw.append(_wrap16(gB[offB[b0]:offB[b1]].reshape(-1)))
        gAw = np.ascontiguousarray(np.concatenate(gAw, axis=1))
        gBw = np.ascontiguousarray(np.concatenate(gBw, axis=1))

        # hT_own: col b*128 + pos = h[node]
        hT_own = np.zeros((P, NODE_PAD), dtype=ml_dtypes.bfloat16)
        own = np.arange(k * NODES_PER_CORE, (k + 1) * NODES_PER_CORE)
        hT_own[:, node_block * P + node_pos] = h[own].T.astype(
            ml_dtypes.bfloat16)

        in_maps.append({
            "hT": hT,
            "hT_own": hT_own,
            "wcat": np.ascontiguousarray(wcat),
            "wad": np.ascontiguousarray(wad),
            "gidxA": gAw,
            "gidxB": gBw,
        })
        # outO row for node (block, pos) = pos*BLOCKS + block
        unpack_maps.append(node_pos * BLOCKS + node_block)

    return in_maps, (KLOs, KHIs, groups), unpack_maps



def _gather_narrow(nc, mybir, out_ap, in_ap, idxs_ap, num_idxs, elem_size,
                   elem_step):
    """dma_gather with elem_size_bytes not a multiple of 256 (the builder's
    %256 assert is a transpose-mode restriction; non-transpose SDMA
    descriptors support arbitrary lengths). Mirrors BassGpSimd.dma_gather's
    lowering for the plain DRAM-source, gen_mode=0 case."""
    g = nc.gpsimd
    stride_bytes = elem_step * mybir.dt.size(in_ap.dtype)
    _in_ap = g.lower_ap_dma(in_ap, for_custom_bir_dma=True)
    _idxs_ap = g.lower_ap(idxs_ap)
    _out_ap = g.lower_ap(out_ap)
    return g.add_instruction(
        mybir.InstDMAGatherAnt(
            name=nc.get_next_instruction_name(),
            ins=[*_in_ap, _idxs_ap, g.lower_val_access(g.to_reg(num_idxs))],
            outs=[_out_ap],
            transpose=False,
            num_idxs=num_idxs,
            elem_size=elem_size,
            stride_bytes_256=stride_bytes // 256,
            gen_mode=0,
            single_packet=True,
            queue_num=0,
            sbuf_tokens_per_rank=0,
            sbuf_free_dim_per_rank=0,
            sbuf_free_dim_pad_per_rank=0,
            sbuf_byte_offset=0,
        ))


def _build_program(plan):
    import concourse.bacc as bacc
    import concourse.tile as tile
    import concourse.mybir as mybir

    KLOs, KHIs, groups = plan
    LA = int(KLOs.sum()) * P
    LB = int(KHIs.sum()) * P
    f32 = mybir.dt.float32
    bf16 = mybir.dt.bfloat16
    i16 = mybir.dt.int16

    import os as _os
    _simclean = _os.environ.get("SIM_CLEAN", "0") == "1"
    nc = bacc.Bacc("TRN2", target_bir_lowering=False, debug=False,
                   dynamic_dma_scratch_size=DMA_SCRATCH)

    hT = nc.dram_tensor("hT", [P, N_NODES_PAD], bf16, kind="ExternalInput")
    hT_own = nc.dram_tensor("hT_own", [P, NODE_PAD], bf16, kind="ExternalInput")
    wcat_d = nc.dram_tensor("wcat", [P, 68], bf16, kind="ExternalInput")
    wad_d = nc.dram_tensor("wad", [P, 4], bf16, kind="ExternalInput")
    gidxA = nc.dram_tensor("gidxA", [P, LA // 16], i16, kind="ExternalInput")
    gidxB = nc.dram_tensor("gidxB", [P, LB // 16], i16, kind="ExternalInput")

    zaug = nc.dram_tensor("zaug", [N_NODES_PAD, 128], bf16)
    outO = nc.dram_tensor("outO", [NODE_PAD, HF], f32, kind="ExternalOutput")

    SC = 4                        # chunks per PSUM tile
    SC2 = 8                       # chunks per load/store superchunk

    with tile.TileContext(nc) as tc:
        with (
            tc.tile_pool(name="const", bufs=1) as cpool,
            tc.tile_pool(name="pa", bufs=8) as pa,
            tc.tile_pool(name="papsum", bufs=4, space="PSUM") as papsum,
            tc.tile_pool(name="adpsum", bufs=1, space="PSUM") as adpsum,
            tc.tile_pool(name="epA", bufs=3) as epA,
            tc.tile_pool(name="epB", bufs=3) as epB,
            tc.tile_pool(name="ep", bufs=4) as ep,
            tc.tile_pool(name="fp", bufs=3) as fp,
        ):
            wcat_t = cpool.tile([P, 68], bf16)
            nc.sync.dma_start(out=wcat_t[:], in_=wcat_d[:])
            wad_t = cpool.tile([P, 4], bf16)
            nc.sync.dma_start(out=wad_t[:], in_=wad_d[:])
            ho_t = cpool.tile([P, NODE_PAD], bf16)
            nc.sync.dma_start(out=ho_t[:], in_=hT_own[:])
            gA_t = cpool.tile([P, LA // 16], i16)
            nc.sync.dma_start(out=gA_t[:], in_=gidxA[:])
            gB_t = cpool.tile([P, LB // 16], i16)
            nc.sync.dma_start(out=gB_t[:], in_=gidxB[:])

            # ---------------- Phase A0: ad for own nodes (SBUF resident) ----
            adp = adpsum.tile([P, BLOCKS, 4], f32)
            for b in range(BLOCKS):
                nc.tensor.matmul(
                    out=adp[:, b, :],
                    lhsT=ho_t[:, b * P:(b + 1) * P],
                    rhs=wad_t[:],
                    start=True, stop=True,
                )
            adall = cpool.tile([P, BLOCKS, 4], f32)
            nc.scalar.copy(out=adall[:], in_=adp[:])

            # ---------------- Phase A: zaug for all nodes -------------------
            n_sc = NCHUNK // SC2
            all_scs = [(s * SC2, SC2) for s in range(n_sc)]
            if NCHUNK % SC2:
                all_scs.append((n_sc * SC2, NCHUNK % SC2))
            z3 = zaug[:].rearrange("(p s) e -> p s e", s=NCHUNK)
            for s0, nsub in all_scs:
                hsl = pa.tile([P, SC2 * P], bf16, tag="hsl")
                nc.sync.dma_start(
                    out=hsl[:, :nsub * P],
                    in_=hT[:, s0 * P:(s0 + nsub) * P])
                zst = pa.tile([P, SC2, 64], f32, tag="zst")
                if _simclean:
                    nc.scalar.memzero(zst[:])
                zbf = zst[:].bitcast(mybir.dt.bfloat16)
                for g0 in range(0, nsub, SC):
                    g1 = min(g0 + SC, nsub)
                    zp = papsum.tile([P, SC, 68], f32, tag="zp")
                    for j in range(g0, g1):
                        nc.tensor.matmul(
                            out=zp[:, j - g0, :],
                            lhsT=hsl[:, j * P:(j + 1) * P],
                            rhs=wcat_t[:],
                            start=True, stop=True,
                        )
                    nc.scalar.copy(out=zbf[:, g0:g1, 0:64],
                                   in_=zp[:, :g1 - g0, 0:64])
                    nc.scalar.copy(out=zst[:, g0:g1, 32:36],
                                   in_=zp[:, :g1 - g0, 64:68])
                # overwrite as = -60 on the reserved pad rows (p, s) =
                # (0, 0) and (96, 0) before the store
                if s0 == 0:
                    nc.vector.memset(zst[0:1, 0, 32:36], -60.0)
                    nc.vector.memset(zst[96:97, 0, 32:36], -60.0)
                nc.sync.dma_start(out=z3[:, s0:s0 + nsub, :],
                                  in_=zst[:].bitcast(bf16)[:, :nsub, :])

            # ---------------- Edge phase ------------------------------------
            viewA = zaug[0:VIEW, 0:GE]
            viewB = zaug[HIB:N_NODES_PAD, 0:GE]
            outS = cpool.tile([P, BLOCKS, HF], f32)
            offA = np.concatenate([[0], np.cumsum(KLOs)])
            offB = np.concatenate([[0], np.cumsum(KHIs)])
            o16A = o16B = 0
            for g in groups:
                b0, b1 = g[0], g[-1] + 1
                ga_ch = int(offA[b1] - offA[b0])
                gb_ch = int(offB[b1] - offB[b0])
                zsA = epA.tile([P, ga_ch, GE], bf16, tag="zsA")
                for c0 in range(0, ga_ch, CALL_CHUNKS):
                    c1 = min(c0 + CALL_CHUNKS, ga_ch)
                    _gather_narrow(
                        nc, mybir,
                        out_ap=zsA[:, c0:c1, :],
                        in_ap=viewA,
                        idxs_ap=gA_t[:, o16A + c0 * 8:o16A + c1 * 8],
                        num_idxs=(c1 - c0) * P,
                        elem_size=GE, elem_step=128,
                    )
                zsB = epB.tile([P, gb_ch, GE], bf16, tag="zsB")
                for c0 in range(0, gb_ch, CALL_CHUNKS):
                    c1 = min(c0 + CALL_CHUNKS, gb_ch)
                    _gather_narrow(
                        nc, mybir,
                        out_ap=zsB[:, c0:c1, :],
                        in_ap=viewB,
                        idxs_ap=gB_t[:, o16B + c0 * 8:o16B + c1 * 8],
                        num_idxs=(c1 - c0) * P,
                        elem_size=GE, elem_step=128,
                    )
                o16A += ga_ch * 8
                o16B += gb_ch * 8
                for b in g:
                    KA, KB = int(KLOs[b]), int(KHIs[b])
                    K = KA + KB
                    ca = int(offA[b] - offA[b0])
                    cb_ = int(offB[b] - offB[b0])
                    adb = adall[:, b, :].unsqueeze(1)
                    et = ep.tile([P, K, 4], f32, tag="et")
                    nc.vector.tensor_add(
                        out=et[:, 0:KA, :],
                        in0=zsA[:].bitcast(f32)[:, ca:ca + KA, 32:36],
                        in1=adb.broadcast_to([P, KA, 4]))
                    nc.vector.tensor_add(
                        out=et[:, KA:K, :],
                        in0=zsB[:].bitcast(f32)[:, cb_:cb_ + KB, 32:36],
                        in1=adb.broadcast_to([P, KB, 4]))
                    ex1 = ep.tile([P, K, 4], bf16, tag="ex1")
                    nc.scalar.activation(ex1[:], et[:],
                                         mybir.ActivationFunctionType.Exp)
                    ext = ep.tile([P, K, 4], bf16, tag="ext")
                    nc.scalar.activation(ext[:], et[:],
                                         mybir.ActivationFunctionType.Exp,
                                         scale=ALPHA)
                    nc.vector.tensor_tensor(out=ext[:], in0=ext[:], in1=ex1[:],
                                            op=mybir.AluOpType.max)
                    # rhs = [ex * zs | ex]
                    rhs_t = ep.tile([P, K, 68], bf16, tag="rhs")
                    exb = ext[:].unsqueeze(3)
                    zsbA = zsA[:, ca:ca + KA, 0:64].rearrange(
                        "p k (h f) -> p k h f", h=4)
                    nc.vector.tensor_tensor(
                        out=rhs_t[:, 0:KA, 0:64].rearrange(
                            "p k (h f) -> p k h f", h=4),
                        in0=zsbA,
                        in1=exb[:, 0:KA, :, :].broadcast_to([P, KA, 4, 16]),
                        op=mybir.AluOpType.mult)
                    zsbB = zsB[:, cb_:cb_ + KB, 0:64].rearrange(
                        "p k (h f) -> p k h f", h=4)
                    nc.vector.tensor_tensor(
                        out=rhs_t[:, KA:K, 0:64].rearrange(
                            "p k (h f) -> p k h f", h=4),
                        in0=zsbB,
                        in1=exb[:, KA:K, :, :].broadcast_to([P, KB, 4, 16]),
                        op=mybir.AluOpType.mult)
                    nc.scalar.copy(out=rhs_t[:, :, 64:68], in_=ext[:])
                    # [num | denom]: pairwise tree-sum over chunks (all
                    # operands contiguous, f32 accumulation)
                    n = K
                    while n > 1:
                        hh = n // 2
                        nc.vector.tensor_add(
                            out=rhs_t[:, 0:hh, :],
                            in0=rhs_t[:, 0:hh, :],
                            in1=rhs_t[:, n - hh:n, :])
                        n -= hh
                    red = rhs_t[:, 0, :]
                    rec = fp.tile([P, 4], f32, tag="rec")
                    nc.vector.reciprocal(rec[:], red[:, 64:68])
                    nc.vector.tensor_tensor(
                        out=outS[:, b, :].rearrange("p (h f) -> p h f", h=4),
                        in0=red[:, 0:64].rearrange("p (h f) -> p h f", h=4),
                        in1=rec[:].unsqueeze(2).broadcast_to([P, 4, 16]),
                        op=mybir.AluOpType.mult)

            o3 = outO[:].rearrange("(p s) e -> p s e", s=BLOCKS)
            nc.sync.dma_start(out=o3[:], in_=outS[:])

    nc.finalize()
    return nc


def kernel(h, src, dst, W, A):
    from concourse.bass_utils import run_bass_kernel_spmd

    in_maps, plan, unpack_maps = _build_host_data(h, src, dst, W, A)
    nc = _build_program(plan)
    res = run_bass_kernel_spmd(nc, in_maps, core_ids=list(range(N_CORES)))
    out = np.empty((N_NODES, HF), dtype=np.float32)
    for k in range(N_CORES):
        out[k * NODES_PER_CORE:(k + 1) * NODES_PER_CORE] = \
            res.results[k]["outO"][unpack_maps[k]]
    return out


# revision 29
# speedup vs baseline: 1.3334x; 1.0169x over previous
"""GAT layer (nn_GATLayer) on 8 Trainium2 NeuronCores via Bass/Tile.

Strategy (dst-partitioned; degree-aligned slots, softmax fully local per core):
  - Core k owns dst nodes [k*6250, (k+1)*6250). Each owned node is pinned to a
    (block, partition) slot; ALL of its in-edges occupy that partition across
    the block's chunks. Segment-softmax then needs no scatter at all: the
    per-node sums are free-axis reductions, and the dst attention term ad is a
    per-partition scalar broadcast.
  - Phase A (replicated): zaug[row] = [z bf16 x64 | as f32 x4 | pad] (256B rows)
    for ALL nodes via one matmul with folded weights [W.T | W.T@A1blk].
    Rows are partition-major (row = p*391 + s) so stores are contiguous.
    Two reserved rows (0 and 50047) get as = -60 patched in: pad slots gather
    them and contribute exp(leaky(-60+ad)) ~ 1e-5 to denom and 0 to num (z=0).
  - Phase A0: ad = hT_own @ (W.T@A2blk) for own nodes in (block, pos) order;
    stays SBUF-resident [128, 49, 4].
  - Edge phase per block: dma_gather of zaug rows by src (two overlapping
    int16 views: A = rows [0, 32768), B = rows [17280, 50048); per-node lo/hi
    edge split chosen on host), e = as + ad, ex = max(exp(e), exp(0.2e)),
    rhs = [ex*zs | ex], then one reduce_sum over chunks -> [num | denom];
    out = num * recip(denom), accumulated in SBUF, single store at the end.
  - Gather calls span several blocks (SWDGE ring enlarged to 4096 descs) to
    amortize the per-call desc-gen overhead on Pool.

All index/layout prep (row permutation placing high-out-degree nodes in the
overlapping view region, per-core 2D block packing, per-block KLO/KHI chunk
budgets uniformized across cores so one program serves all 8) is done on host.
"""

import numpy as np

N_NODES = 50000
N_EDGES = 800000
IN_FEATS = 128
OUT_FEATS = 16
NUM_HEADS = 4
ALPHA = 0.2
HF = NUM_HEADS * OUT_FEATS  # 64

N_CORES = 8
P = 128
NODES_PER_CORE = N_NODES // N_CORES     # 6250
BLOCKS = 49                              # ceil(6250/128)
NODE_PAD = BLOCKS * P                    # 6272
NCHUNK = 391                             # zaug chunks; 128*391 = 50048 rows
N_NODES_PAD = P * NCHUNK                 # 50048
VIEW = 32768                             # int16 gather view size
HIB = N_NODES_PAD - VIEW                 # 17280 = base of view B
PAD_A_ROW = 0                            # reserved pad row in view A (p0, s0)
PAD_B_ROW = 96 * NCHUNK                  # reserved pad row in view B (p96, s0)
DMA_SCRATCH = 16384                      # SWDGE ring: 1024 descriptors (HW cap per call)
CALL_CHUNKS = 8                          # max chunks (128 idx each) per gather
TILE_CHUNKS = 48                         # max chunks per stream per zs tile
GE = 72                                  # gather elem: 72 bf16 = 144B of each 256B row


def _wrap16(vals):
    # gather idx layout: stream position i -> idx tile [16, n/16] at
    # [i%16, i//16]; rows replicated to 128 partitions.
    n = vals.shape[-1]
    w = vals.reshape(n // 16, 16).T                    # [16, n/16]
    return np.tile(w, (8, 1))                          # [128, n/16]


def _plan_groups(klos, khis):
    """Greedy grouping of consecutive blocks into zs-tile groups with
    sum(KLO) <= cap and sum(KHI) <= cap (single blocks may exceed the cap;
    their gathers are split into CALL_CHUNKS-sized calls). The last ~20% of
    chunks use smaller groups so the trailing compute tail is short."""
    tot = int(klos.sum() + khis.sum())
    groups = []
    cur = []
    sa = sb = done = 0
    for b in range(len(klos)):
        ka, kb = klos[b], khis[b]
        cap = TILE_CHUNKS if done < 0.8 * tot else TILE_CHUNKS // 3
        if cur and (sa + ka > cap or sb + kb > cap):
            groups.append(cur)
            cur, sa, sb = [], 0, 0
        cur.append(b)
        sa += ka
        sb += kb
        done += ka + kb
    if cur:
        groups.append(cur)
    return groups


def _build_host_data(h, src, dst, W, A):
    import ml_dtypes

    src = np.asarray(src)
    dst = np.asarray(dst)
    W = np.asarray(W, dtype=np.float32)
    A = np.asarray(A, dtype=np.float32)
    h = np.asarray(h, dtype=np.float32)

    # folded weights
    A1blk = np.zeros((HF, NUM_HEADS), dtype=np.float32)
    A2blk = np.zeros((HF, NUM_HEADS), dtype=np.float32)
    for hd in range(NUM_HEADS):
        A1blk[hd * OUT_FEATS:(hd + 1) * OUT_FEATS, hd] = A[hd, :OUT_FEATS]
        A2blk[hd * OUT_FEATS:(hd + 1) * OUT_FEATS, hd] = A[hd, OUT_FEATS:]
    WT = np.ascontiguousarray(W.T)                                  # [128, 64]
    wcat = np.concatenate([WT, WT @ A1blk], axis=1).astype(ml_dtypes.bfloat16)
    wad = (WT @ A2blk).astype(ml_dtypes.bfloat16)                   # [128, 4]

    # global row permutation: high-out-degree nodes -> overlap rows
    # [HIB, VIEW); rows 0 and 50047 reserved for pad targets.
    outdeg = np.bincount(src, minlength=N_NODES)
    nodes_by_heat = np.argsort(-outdeg, kind="stable")
    ov_rows = np.arange(HIB, VIEW)
    rest_hi = np.arange(VIEW, N_NODES_PAD)
    rest = np.concatenate(
        [np.arange(1, HIB), rest_hi[rest_hi != PAD_B_ROW]])
    perm = np.empty(N_NODES, dtype=np.int64)
    perm[nodes_by_heat[:len(ov_rows)]] = ov_rows
    perm[nodes_by_heat[len(ov_rows):]] = rest[:N_NODES - len(ov_rows)]

    # hT column for row r: phase A chunk s partition p -> row p*391 + s,
    # processed from hT col s*128 + p.
    hT = np.zeros((P, N_NODES_PAD), dtype=ml_dtypes.bfloat16)
    cols = (perm % NCHUNK) * P + perm // NCHUNK
    hT[:, cols] = h.T.astype(ml_dtypes.bfloat16)

    # per-core edge prep
    order = np.argsort(dst, kind="stable")
    dst_s = dst[order]
    rows_s = perm[src[order]]
    core_begin = np.searchsorted(
        dst_s, np.arange(0, N_NODES + 1, NODES_PER_CORE))

    cores = []
    for k in range(N_CORES):
        lo_e, hi_e = core_begin[k], core_begin[k + 1]
        cd = dst_s[lo_e:hi_e] - k * NODES_PER_CORE
        rw = rows_s[lo_e:hi_e]
        is_a = rw < HIB                      # A-only
        is_b = rw >= VIEW                    # B-only
        is_f = ~is_a & ~is_b                 # flexible
        a = np.bincount(cd[is_a], minlength=NODES_PER_CORE)
        c = np.bincount(cd[is_b], minlength=NODES_PER_CORE)
        f = np.bincount(cd[is_f], minlength=NODES_PER_CORE)
        T = a + c + f
        node_order = np.lexsort((a - c, (-T) // 3))
        # node -> (block, pos)
        node_block = np.empty(NODES_PER_CORE, dtype=np.int64)
        node_pos = np.empty(NODES_PER_CORE, dtype=np.int64)
        node_block[node_order] = np.arange(NODES_PER_CORE) // P
        node_pos[node_order] = np.arange(NODES_PER_CORE) % P
        # per-block requirement maxima (budget components)
        amax = np.zeros(BLOCKS, dtype=np.int64)
        cmax = np.zeros(BLOCKS, dtype=np.int64)
        tmax = np.zeros(BLOCKS, dtype=np.int64)
        for b in range(BLOCKS):
            blk = node_order[b * P:(b + 1) * P]
            amax[b] = a[blk].max()
            cmax[b] = c[blk].max()
            tmax[b] = T[blk].max()
        cores.append(dict(cd=cd, rw=rw, a=a, c=c, f=f, T=T,
                          node_block=node_block, node_pos=node_pos,
                          amax=amax, cmax=cmax, tmax=tmax))

    # minimal uniform per-block chunk budgets across cores: any split with
    # KLO >= max a, KHI >= max c, KLO+KHI >= max T is feasible per node.
    Astar = np.max([co["amax"] for co in cores], axis=0)
    Cstar = np.max([co["cmax"] for co in cores], axis=0)
    Tstar = np.max([co["tmax"] for co in cores], axis=0)
    Kb = np.maximum(Tstar, Astar + Cstar)
    KLOs = np.maximum(Astar, Kb - Cstar)
    KHIs = Kb - KLOs
    groups = _plan_groups(KLOs, KHIs)
    LA = int(KLOs.sum()) * P
    LB = int(KHIs.sum()) * P
    offA = np.concatenate([[0], np.cumsum(KLOs)])    # chunk offsets per block
    offB = np.concatenate([[0], np.cumsum(KHIs)])

    in_maps = []
    unpack_maps = []
    for k in range(N_CORES):
        co = cores[k]
        cd, rw = co["cd"], co["rw"]
        a, f, T = co["a"], co["f"], co["T"]
        node_block, node_pos = co["node_block"], co["node_pos"]
        # per-node lo count: L = max(a, T - KHI_block)
        KHI_n = KHIs[node_block]
        L = np.maximum(a, T - KHI_n)

        # sort edges by (node, flexibility-class) so each node's edge list is
        # [A-only..., flex..., B-only...]; first L edges -> stream A.
        cls = np.where(rw < HIB, 0, np.where(rw < VIEW, 1, 2))
        eo = np.lexsort((cls, cd))
        cd_o, rw_o = cd[eo], rw[eo]
        starts = np.searchsorted(cd_o, np.arange(NODES_PER_CORE + 1))
        rank = np.arange(len(cd_o)) - starts[cd_o]          # rank within node
        to_a = rank < L[cd_o]

        gA = np.full((LA // P, P), PAD_A_ROW, dtype=np.int16)
        gB = np.full((LB // P, P), PAD_B_ROW - HIB, dtype=np.int16)
        # slot chunk = offX[block] + rank (A) or rank - L (B)
        blk_e = node_block[cd_o]
        pos_e = node_pos[cd_o]
        ca = offA[blk_e] + rank
        cb_ = offB[blk_e] + rank - L[cd_o]
        gA[ca[to_a], pos_e[to_a]] = rw_o[to_a].astype(np.int16)
        gB[cb_[~to_a], pos_e[~to_a]] = (rw_o[~to_a] - HIB).astype(np.int16)

        # wrap16 per call group
        gAw, gBw = [], []
        for g in groups:
            b0, b1 = g[0], g[-1] + 1
            gAw.append(_wrap16(gA[offA[b0]:offA[b1]].reshape(-1)))
            gBw.append(_wrap16(gB[offB[b0]:offB[b1]].reshape(-1)))
        gAw = np.ascontiguousarray(np.concatenate(gAw, axis=1))
        gBw = np.ascontiguousarray(np.concatenate(gBw, axis=1))

        # hT_own: col b*128 + pos = h[node]
        hT_own = np.zeros((P, NODE_PAD), dtype=ml_dtypes.bfloat16)
        own = np.arange(k * NODES_PER_CORE, (k + 1) * NODES_PER_CORE)
        hT_own[:, node_block * P + node_pos] = h[own].T.astype(
            ml_dtypes.bfloat16)

        in_maps.append({
            "hT": hT,
            "hT_own": hT_own,
            "wcat": np.ascontiguousarray(wcat),
            "wad": np.ascontiguousarray(wad),
            "gidxA": gAw,
            "gidxB": gBw,
        })
        # outO row for node (block, pos) = pos*BLOCKS + block
        unpack_maps.append(node_pos * BLOCKS + node_block)

    return in_maps, (KLOs, KHIs, groups), unpack_maps



def _gather_narrow(nc, mybir, out_ap, in_ap, idxs_ap, num_idxs, elem_size,
                   elem_step):
    """dma_gather with elem_size_bytes not a multiple of 256 (the builder's
    %256 assert is a transpose-mode restriction; non-transpose SDMA
    descriptors support arbitrary lengths). Mirrors BassGpSimd.dma_gather's
    lowering for the plain DRAM-source, gen_mode=0 case."""
    g = nc.gpsimd
    stride_bytes = elem_step * mybir.dt.size(in_ap.dtype)
    _in_ap = g.lower_ap_dma(in_ap, for_custom_bir_dma=True)
    _idxs_ap = g.lower_ap(idxs_ap)
    _out_ap = g.lower_ap(out_ap)
    return g.add_instruction(
        mybir.InstDMAGatherAnt(
            name=nc.get_next_instruction_name(),
            ins=[*_in_ap, _idxs_ap, g.lower_val_access(g.to_reg(num_idxs))],
            outs=[_out_ap],
            transpose=False,
            num_idxs=num_idxs,
            elem_size=elem_size,
            stride_bytes_256=stride_bytes // 256,
            gen_mode=0,
            single_packet=True,
            queue_num=0,
            sbuf_tokens_per_rank=0,
            sbuf_free_dim_per_rank=0,
            sbuf_free_dim_pad_per_rank=0,
            sbuf_byte_offset=0,
        ))


def _build_program(plan):
    import concourse.bacc as bacc
    import concourse.tile as tile
    import concourse.mybir as mybir

    KLOs, KHIs, groups = plan
    LA = int(KLOs.sum()) * P
    LB = int(KHIs.sum()) * P
    f32 = mybir.dt.float32
    bf16 = mybir.dt.bfloat16
    i16 = mybir.dt.int16

    import os as _os
    _simclean = _os.environ.get("SIM_CLEAN", "0") == "1"
    nc = bacc.Bacc("TRN2", target_bir_lowering=False, debug=False,
                   dynamic_dma_scratch_size=DMA_SCRATCH)

    hT = nc.dram_tensor("hT", [P, N_NODES_PAD], bf16, kind="ExternalInput")
    hT_own = nc.dram_tensor("hT_own", [P, NODE_PAD], bf16, kind="ExternalInput")
    wcat_d = nc.dram_tensor("wcat", [P, 68], bf16, kind="ExternalInput")
    wad_d = nc.dram_tensor("wad", [P, 4], bf16, kind="ExternalInput")
    gidxA = nc.dram_tensor("gidxA", [P, LA // 16], i16, kind="ExternalInput")
    gidxB = nc.dram_tensor("gidxB", [P, LB // 16], i16, kind="ExternalInput")

    zaug = nc.dram_tensor("zaug", [N_NODES_PAD, 128], bf16)
    outO = nc.dram_tensor("outO", [NODE_PAD, HF], f32, kind="ExternalOutput")

    SC = 4                        # chunks per PSUM tile
    SC2 = 16                      # chunks per load/store superchunk

    with tile.TileContext(nc) as tc:
        with (
            tc.tile_pool(name="const", bufs=1) as cpool,
            tc.tile_pool(name="pa", bufs=8) as pa,
            tc.tile_pool(name="papsum", bufs=4, space="PSUM") as papsum,
            tc.tile_pool(name="adpsum", bufs=1, space="PSUM") as adpsum,
            tc.tile_pool(name="epA", bufs=3) as epA,
            tc.tile_pool(name="epB", bufs=3) as epB,
            tc.tile_pool(name="ep", bufs=4) as ep,
            tc.tile_pool(name="fp", bufs=3) as fp,
        ):
            wcat_t = cpool.tile([P, 68], bf16)
            nc.sync.dma_start(out=wcat_t[:], in_=wcat_d[:])
            gA_t = cpool.tile([P, LA // 16], i16)
            nc.sync.dma_start(out=gA_t[:], in_=gidxA[:])

            # ---------------- Phase A: zaug for all nodes -------------------
            n_sc = NCHUNK // SC2
            all_scs = [(s * SC2, SC2) for s in range(n_sc)]
            if NCHUNK % SC2:
                all_scs.append((n_sc * SC2, NCHUNK % SC2))
            z3 = zaug[:].rearrange("(p s) e -> p s e", s=NCHUNK)
            for s0, nsub in all_scs:
                hsl = pa.tile([P, SC2 * P], bf16, tag="hsl")
                nc.sync.dma_start(
                    out=hsl[:, :nsub * P],
                    in_=hT[:, s0 * P:(s0 + nsub) * P])
                zst = pa.tile([P, SC2, 64], f32, tag="zst")
                if _simclean:
                    nc.scalar.memzero(zst[:])
                zbf = zst[:].bitcast(mybir.dt.bfloat16)
                for g0 in range(0, nsub, SC):
                    g1 = min(g0 + SC, nsub)
                    zp = papsum.tile([P, SC, 68], f32, tag="zp")
                    for j in range(g0, g1):
                        nc.tensor.matmul(
                            out=zp[:, j - g0, :],
                            lhsT=hsl[:, j * P:(j + 1) * P],
                            rhs=wcat_t[:],
                            start=True, stop=True,
                        )
                    nc.scalar.copy(out=zbf[:, g0:g1, 0:64],
                                   in_=zp[:, :g1 - g0, 0:64])
                    nc.scalar.copy(out=zst[:, g0:g1, 32:36],
                                   in_=zp[:, :g1 - g0, 64:68])
                # overwrite as = -60 on the reserved pad rows (p, s) =
                # (0, 0) and (96, 0) before the store
                if s0 == 0:
                    nc.vector.memset(zst[0:1, 0, 32:36], -60.0)
                    nc.vector.memset(zst[96:97, 0, 32:36], -60.0)
                nc.sync.dma_start(out=z3[:, s0:s0 + nsub, :],
                                  in_=zst[:].bitcast(bf16)[:, :nsub, :])

            # ---------------- Phase A0 + remaining loads (overlap edge) -----
            wad_t = cpool.tile([P, 4], bf16)
            nc.sync.dma_start(out=wad_t[:], in_=wad_d[:])
            ho_t = cpool.tile([P, NODE_PAD], bf16)
            nc.sync.dma_start(out=ho_t[:], in_=hT_own[:])
            gB_t = cpool.tile([P, LB // 16], i16)
            nc.sync.dma_start(out=gB_t[:], in_=gidxB[:])
            adp = adpsum.tile([P, BLOCKS, 4], f32)
            for b in range(BLOCKS):
                nc.tensor.matmul(
                    out=adp[:, b, :],
                    lhsT=ho_t[:, b * P:(b + 1) * P],
                    rhs=wad_t[:],
                    start=True, stop=True,
                )
            adall = cpool.tile([P, BLOCKS, 4], f32)
            nc.scalar.copy(out=adall[:], in_=adp[:])

            # ---------------- Edge phase ------------------------------------
            viewA = zaug[0:VIEW, 0:GE]
            viewB = zaug[HIB:N_NODES_PAD, 0:GE]
            outS = cpool.tile([P, BLOCKS, HF], f32)
            o3 = outO[:].rearrange("(p s) e -> p s e", s=BLOCKS)
            offA = np.concatenate([[0], np.cumsum(KLOs)])
            offB = np.concatenate([[0], np.cumsum(KHIs)])
            o16A = o16B = 0
            for g in groups:
                b0, b1 = g[0], g[-1] + 1
                ga_ch = int(offA[b1] - offA[b0])
                gb_ch = int(offB[b1] - offB[b0])
                zsA = epA.tile([P, ga_ch, GE], bf16, tag="zsA")
                for c0 in range(0, ga_ch, CALL_CHUNKS):
                    c1 = min(c0 + CALL_CHUNKS, ga_ch)
                    _gather_narrow(
                        nc, mybir,
                        out_ap=zsA[:, c0:c1, :],
                        in_ap=viewA,
                        idxs_ap=gA_t[:, o16A + c0 * 8:o16A + c1 * 8],
                        num_idxs=(c1 - c0) * P,
                        elem_size=GE, elem_step=128,
                    )
                zsB = epB.tile([P, gb_ch, GE], bf16, tag="zsB")
                for c0 in range(0, gb_ch, CALL_CHUNKS):
                    c1 = min(c0 + CALL_CHUNKS, gb_ch)
                    _gather_narrow(
                        nc, mybir,
                        out_ap=zsB[:, c0:c1, :],
                        in_ap=viewB,
                        idxs_ap=gB_t[:, o16B + c0 * 8:o16B + c1 * 8],
                        num_idxs=(c1 - c0) * P,
                        elem_size=GE, elem_step=128,
                    )
                o16A += ga_ch * 8
                o16B += gb_ch * 8
                for b in g:
                    KA, KB = int(KLOs[b]), int(KHIs[b])
                    K = KA + KB
                    ca = int(offA[b] - offA[b0])
                    cb_ = int(offB[b] - offB[b0])
                    adb = adall[:, b, :].unsqueeze(1)
                    et = ep.tile([P, K, 4], f32, tag="et")
                    nc.vector.tensor_add(
                        out=et[:, 0:KA, :],
                        in0=zsA[:].bitcast(f32)[:, ca:ca + KA, 32:36],
                        in1=adb.broadcast_to([P, KA, 4]))
                    nc.vector.tensor_add(
                        out=et[:, KA:K, :],
                        in0=zsB[:].bitcast(f32)[:, cb_:cb_ + KB, 32:36],
                        in1=adb.broadcast_to([P, KB, 4]))
                    ex1 = ep.tile([P, K, 4], bf16, tag="ex1")
                    nc.scalar.activation(ex1[:], et[:],
                                         mybir.ActivationFunctionType.Exp)
                    ext = ep.tile([P, K, 4], bf16, tag="ext")
                    nc.scalar.activation(ext[:], et[:],
                                         mybir.ActivationFunctionType.Exp,
                                         scale=ALPHA)
                    nc.vector.tensor_tensor(out=ext[:], in0=ext[:], in1=ex1[:],
                                            op=mybir.AluOpType.max)
                    # rhs = [ex * zs | ex]
                    rhs_t = ep.tile([P, K, 68], bf16, tag="rhs")
                    exb = ext[:].unsqueeze(3)
                    zsbA = zsA[:, ca:ca + KA, 0:64].rearrange(
                        "p k (h f) -> p k h f", h=4)
                    nc.vector.tensor_tensor(
                        out=rhs_t[:, 0:KA, 0:64].rearrange(
                            "p k (h f) -> p k h f", h=4),
                        in0=zsbA,
                        in1=exb[:, 0:KA, :, :].broadcast_to([P, KA, 4, 16]),
                        op=mybir.AluOpType.mult)
                    zsbB = zsB[:, cb_:cb_ + KB, 0:64].rearrange(
                        "p k (h f) -> p k h f", h=4)
                    nc.vector.tensor_tensor(
                        out=rhs_t[:, KA:K, 0:64].rearrange(
                            "p k (h f) -> p k h f", h=4),
                        in0=zsbB,
                        in1=exb[:, KA:K, :, :].broadcast_to([P, KB, 4, 16]),
                        op=mybir.AluOpType.mult)
                    nc.scalar.copy(out=rhs_t[:, :, 64:68], in_=ext[:])
                    # [num | denom]: pairwise tree-sum over chunks (all
                    # operands contiguous, f32 accumulation)
                    n = K
                    while n > 1:
                        hh = n // 2
                        nc.vector.tensor_add(
                            out=rhs_t[:, 0:hh, :],
                            in0=rhs_t[:, 0:hh, :],
                            in1=rhs_t[:, n - hh:n, :])
                        n -= hh
                    red = rhs_t[:, 0, :]
                    rec = fp.tile([P, 4], f32, tag="rec")
                    nc.vector.reciprocal(rec[:], red[:, 64:68])
                    nc.vector.tensor_tensor(
                        out=outS[:, b, :].rearrange("p (h f) -> p h f", h=4),
                        in0=red[:, 0:64].rearrange("p (h f) -> p h f", h=4),
                        in1=rec[:].unsqueeze(2).broadcast_to([P, 4, 16]),
                        op=mybir.AluOpType.mult)
                nc.sync.dma_start(out=o3[:, b0:b1, :], in_=outS[:, b0:b1, :])



    nc.finalize()
    return nc


def kernel(h, src, dst, W, A):
    from concourse.bass_utils import run_bass_kernel_spmd

    in_maps, plan, unpack_maps = _build_host_data(h, src, dst, W, A)
    nc = _build_program(plan)
    res = run_bass_kernel_spmd(nc, in_maps, core_ids=list(range(N_CORES)))
    out = np.empty((N_NODES, HF), dtype=np.float32)
    for k in range(N_CORES):
        out[k * NODES_PER_CORE:(k + 1) * NODES_PER_CORE] = \
            res.results[k]["outO"][unpack_maps[k]]
    return out


# revision 33
# speedup vs baseline: 1.3690x; 1.0267x over previous
"""GAT layer (nn_GATLayer) on 8 Trainium2 NeuronCores via Bass/Tile.

Strategy (dst-partitioned; degree-aligned slots, softmax fully local per core):
  - Core k owns dst nodes [k*6250, (k+1)*6250). Each owned node is pinned to a
    (block, partition) slot; ALL of its in-edges occupy that partition across
    the block's chunks. Segment-softmax then needs no scatter at all: the
    per-node sums are free-axis reductions, and the dst attention term ad is a
    per-partition scalar broadcast.
  - Phase A (replicated): zaug[row] = [z bf16 x64 | as f32 x4 | pad] (256B rows)
    for ALL nodes via one matmul with folded weights [W.T | W.T@A1blk].
    Rows are partition-major (row = p*391 + s) so stores are contiguous.
    Two reserved rows (0 and 50047) get as = -60 patched in: pad slots gather
    them and contribute exp(leaky(-60+ad)) ~ 1e-5 to denom and 0 to num (z=0).
  - Phase A0: ad = hT_own @ (W.T@A2blk) for own nodes in (block, pos) order;
    stays SBUF-resident [128, 49, 4].
  - Edge phase per block: dma_gather of zaug rows by src (two overlapping
    int16 views: A = rows [0, 32768), B = rows [17280, 50048); per-node lo/hi
    edge split chosen on host), e = as + ad, ex = max(exp(e), exp(0.2e)),
    rhs = [ex*zs | ex], then one reduce_sum over chunks -> [num | denom];
    out = num * recip(denom), accumulated in SBUF, single store at the end.
  - Gather calls span several blocks (SWDGE ring enlarged to 4096 descs) to
    amortize the per-call desc-gen overhead on Pool.

All index/layout prep (row permutation placing high-out-degree nodes in the
overlapping view region, per-core 2D block packing, per-block KLO/KHI chunk
budgets uniformized across cores so one program serves all 8) is done on host.
"""

import numpy as np

N_NODES = 50000
N_EDGES = 800000
IN_FEATS = 128
OUT_FEATS = 16
NUM_HEADS = 4
ALPHA = 0.2
HF = NUM_HEADS * OUT_FEATS  # 64

N_CORES = 8
P = 128
NODES_PER_CORE = N_NODES // N_CORES     # 6250
BLOCKS = 49                              # ceil(6250/128)
NODE_PAD = BLOCKS * P                    # 6272
NCHUNK = 391                             # zaug chunks; 128*391 = 50048 rows
N_NODES_PAD = P * NCHUNK                 # 50048
VIEW = 32768                             # int16 gather view size
HIB = N_NODES_PAD - VIEW                 # 17280 = base of view B
PAD_A_ROW = 0                            # reserved pad row in view A (p0, s0)
PAD_B_ROW = 96 * NCHUNK                  # reserved pad row in view B (p96, s0)
DMA_SCRATCH = 16384                      # SWDGE ring: 1024 descriptors (HW cap per call)
CALL_CHUNKS = 8                          # max chunks (128 idx each) per gather
TILE_CHUNKS = 48                         # max chunks per stream per zs tile
GE = 72                                  # gather elem: 72 bf16 = 144B of each 256B row


def _wrap16(vals):
    # gather idx layout: stream position i -> idx tile [16, n/16] at
    # [i%16, i//16]; rows replicated to 128 partitions.
    n = vals.shape[-1]
    w = vals.reshape(n // 16, 16).T                    # [16, n/16]
    return np.tile(w, (8, 1))                          # [128, n/16]


def _plan_groups(klos, khis):
    """Partition blocks into consecutive zs-tile groups minimizing the total
    number of gather calls (ceil(sumKLO/CALL) + ceil(sumKHI/CALL) per group),
    groups capped at TILE_CHUNKS per stream (smaller near the end so the
    trailing compute tail stays short)."""
    import math
    nb = len(klos)
    tot = int(klos.sum() + khis.sum())
    pre = np.concatenate([[0], np.cumsum(klos + khis)])
    INF = 10 ** 9

    def cap_at(b):
        return TILE_CHUNKS if pre[b] < 0.8 * tot else TILE_CHUNKS // 3

    # dp[i] = (calls, groups) best for blocks[i:]
    dp = [(INF, INF)] * (nb + 1)
    nxt = [0] * nb
    dp[nb] = (0, 0)
    for i in range(nb - 1, -1, -1):
        sa = sb = 0
        for j in range(i, nb):
            sa += int(klos[j])
            sb += int(khis[j])
            cap = cap_at(i)
            if (j > i) and (sa > cap or sb > cap):
                break
            cost = (math.ceil(sa / CALL_CHUNKS) + math.ceil(sb / CALL_CHUNKS)
                    + dp[j + 1][0], 1 + dp[j + 1][1])
            if cost < dp[i]:
                dp[i] = cost
                nxt[i] = j + 1
    groups = []
    i = 0
    while i < nb:
        groups.append(list(range(i, nxt[i])))
        i = nxt[i]
    return groups


def _build_host_data(h, src, dst, W, A):
    import ml_dtypes

    src = np.asarray(src)
    dst = np.asarray(dst)
    W = np.asarray(W, dtype=np.float32)
    A = np.asarray(A, dtype=np.float32)
    h = np.asarray(h, dtype=np.float32)

    # folded weights
    A1blk = np.zeros((HF, NUM_HEADS), dtype=np.float32)
    A2blk = np.zeros((HF, NUM_HEADS), dtype=np.float32)
    for hd in range(NUM_HEADS):
        A1blk[hd * OUT_FEATS:(hd + 1) * OUT_FEATS, hd] = A[hd, :OUT_FEATS]
        A2blk[hd * OUT_FEATS:(hd + 1) * OUT_FEATS, hd] = A[hd, OUT_FEATS:]
    WT = np.ascontiguousarray(W.T)                                  # [128, 64]
    wcat = np.concatenate([WT, WT @ A1blk], axis=1).astype(ml_dtypes.bfloat16)
    wad = (WT @ A2blk).astype(ml_dtypes.bfloat16)                   # [128, 4]

    # global row permutation: high-out-degree nodes -> overlap rows
    # [HIB, VIEW); rows 0 and 50047 reserved for pad targets.
    outdeg = np.bincount(src, minlength=N_NODES)
    nodes_by_heat = np.argsort(-outdeg, kind="stable")
    ov_rows = np.arange(HIB, VIEW)
    rest_hi = np.arange(VIEW, N_NODES_PAD)
    rest = np.concatenate(
        [np.arange(1, HIB), rest_hi[rest_hi != PAD_B_ROW]])
    perm = np.empty(N_NODES, dtype=np.int64)
    perm[nodes_by_heat[:len(ov_rows)]] = ov_rows
    perm[nodes_by_heat[len(ov_rows):]] = rest[:N_NODES - len(ov_rows)]

    # hT column for row r: phase A chunk s partition p -> row p*391 + s,
    # processed from hT col s*128 + p.
    hT = np.zeros((P, N_NODES_PAD), dtype=ml_dtypes.bfloat16)
    cols = (perm % NCHUNK) * P + perm // NCHUNK
    hT[:, cols] = h.T.astype(ml_dtypes.bfloat16)

    # per-core edge prep
    order = np.argsort(dst, kind="stable")
    dst_s = dst[order]
    rows_s = perm[src[order]]
    core_begin = np.searchsorted(
        dst_s, np.arange(0, N_NODES + 1, NODES_PER_CORE))

    cores = []
    for k in range(N_CORES):
        lo_e, hi_e = core_begin[k], core_begin[k + 1]
        cd = dst_s[lo_e:hi_e] - k * NODES_PER_CORE
        rw = rows_s[lo_e:hi_e]
        is_a = rw < HIB                      # A-only
        is_b = rw >= VIEW                    # B-only
        is_f = ~is_a & ~is_b                 # flexible
        a = np.bincount(cd[is_a], minlength=NODES_PER_CORE)
        c = np.bincount(cd[is_b], minlength=NODES_PER_CORE)
        f = np.bincount(cd[is_f], minlength=NODES_PER_CORE)
        T = a + c + f
        node_order = np.lexsort((a - c, (-T) // 3))
        # node -> (block, pos)
        node_block = np.empty(NODES_PER_CORE, dtype=np.int64)
        node_pos = np.empty(NODES_PER_CORE, dtype=np.int64)
        node_block[node_order] = np.arange(NODES_PER_CORE) // P
        node_pos[node_order] = np.arange(NODES_PER_CORE) % P
        # per-block requirement maxima (budget components)
        amax = np.zeros(BLOCKS, dtype=np.int64)
        cmax = np.zeros(BLOCKS, dtype=np.int64)
        tmax = np.zeros(BLOCKS, dtype=np.int64)
        for b in range(BLOCKS):
            blk = node_order[b * P:(b + 1) * P]
            amax[b] = a[blk].max()
            cmax[b] = c[blk].max()
            tmax[b] = T[blk].max()
        cores.append(dict(cd=cd, rw=rw, a=a, c=c, f=f, T=T,
                          node_block=node_block, node_pos=node_pos,
                          amax=amax, cmax=cmax, tmax=tmax))

    # minimal uniform per-block chunk budgets across cores: any split with
    # KLO >= max a, KHI >= max c, KLO+KHI >= max T is feasible per node.
    Astar = np.max([co["amax"] for co in cores], axis=0)
    Cstar = np.max([co["cmax"] for co in cores], axis=0)
    Tstar = np.max([co["tmax"] for co in cores], axis=0)
    Kb = np.maximum(Tstar, Astar + Cstar)
    KLOs = np.maximum(Astar, Kb - Cstar)
    KHIs = Kb - KLOs
    groups = _plan_groups(KLOs, KHIs)
    LA = int(KLOs.sum()) * P
    LB = int(KHIs.sum()) * P
    offA = np.concatenate([[0], np.cumsum(KLOs)])    # chunk offsets per block
    offB = np.concatenate([[0], np.cumsum(KHIs)])

    in_maps = []
    unpack_maps = []
    for k in range(N_CORES):
        co = cores[k]
        cd, rw = co["cd"], co["rw"]
        a, f, T = co["a"], co["f"], co["T"]
        node_block, node_pos = co["node_block"], co["node_pos"]
        # per-node lo count: L = max(a, T - KHI_block)
        KHI_n = KHIs[node_block]
        L = np.maximum(a, T - KHI_n)

        # sort edges by (node, flexibility-class) so each node's edge list is
        # [A-only..., flex..., B-only...]; first L edges -> stream A.
        cls = np.where(rw < HIB, 0, np.where(rw < VIEW, 1, 2))
        eo = np.lexsort((cls, cd))
        cd_o, rw_o = cd[eo], rw[eo]
        starts = np.searchsorted(cd_o, np.arange(NODES_PER_CORE + 1))
        rank = np.arange(len(cd_o)) - starts[cd_o]          # rank within node
        to_a = rank < L[cd_o]

        gA = np.full((LA // P, P), PAD_A_ROW, dtype=np.int16)
        gB = np.full((LB // P, P), PAD_B_ROW - HIB, dtype=np.int16)
        # slot chunk = offX[block] + rank (A) or rank - L (B)
        blk_e = node_block[cd_o]
        pos_e = node_pos[cd_o]
        ca = offA[blk_e] + rank
        cb_ = offB[blk_e] + rank - L[cd_o]
        gA[ca[to_a], pos_e[to_a]] = rw_o[to_a].astype(np.int16)
        gB[cb_[~to_a], pos_e[~to_a]] = (rw_o[~to_a] - HIB).astype(np.int16)

        # wrap16 per call group
        gAw, gBw = [], []
        for g in groups:
            b0, b1 = g[0], g[-1] + 1
            gAw.append(_wrap16(gA[offA[b0]:offA[b1]].reshape(-1)))
            gBw.append(_wrap16(gB[offB[b0]:offB[b1]].reshape(-1)))
        gAw = np.ascontiguousarray(np.concatenate(gAw, axis=1))
        gBw = np.ascontiguousarray(np.concatenate(gBw, axis=1))

        # hT_own: col b*128 + pos = h[node]
        hT_own = np.zeros((P, NODE_PAD), dtype=ml_dtypes.bfloat16)
        own = np.arange(k * NODES_PER_CORE, (k + 1) * NODES_PER_CORE)
        hT_own[:, node_block * P + node_pos] = h[own].T.astype(
            ml_dtypes.bfloat16)

        in_maps.append({
            "hT": hT,
            "hT_own": hT_own,
            "wcat": np.ascontiguousarray(wcat),
            "wad": np.ascontiguousarray(wad),
            "gidxA": gAw,
            "gidxB": gBw,
        })
        # outO row for node (block, pos) = pos*BLOCKS + block
        unpack_maps.append(node_pos * BLOCKS + node_block)

    return in_maps, (KLOs, KHIs, groups), unpack_maps



def _gather_narrow(nc, mybir, out_ap, in_ap, idxs_ap, num_idxs, elem_size,
                   elem_step):
    """dma_gather with elem_size_bytes not a multiple of 256 (the builder's
    %256 assert is a transpose-mode restriction; non-transpose SDMA
    descriptors support arbitrary lengths). Mirrors BassGpSimd.dma_gather's
    lowering for the plain DRAM-source, gen_mode=0 case."""
    g = nc.gpsimd
    stride_bytes = elem_step * mybir.dt.size(in_ap.dtype)
    _in_ap = g.lower_ap_dma(in_ap, for_custom_bir_dma=True)
    _idxs_ap = g.lower_ap(idxs_ap)
    _out_ap = g.lower_ap(out_ap)
    return g.add_instruction(
        mybir.InstDMAGatherAnt(
            name=nc.get_next_instruction_name(),
            ins=[*_in_ap, _idxs_ap, g.lower_val_access(g.to_reg(num_idxs))],
            outs=[_out_ap],
            transpose=False,
            num_idxs=num_idxs,
            elem_size=elem_size,
            stride_bytes_256=stride_bytes // 256,
            gen_mode=0,
            single_packet=True,
            queue_num=0,
            sbuf_tokens_per_rank=0,
            sbuf_free_dim_per_rank=0,
            sbuf_free_dim_pad_per_rank=0,
            sbuf_byte_offset=0,
        ))


def _build_program(plan):
    import concourse.bacc as bacc
    import concourse.tile as tile
    import concourse.mybir as mybir

    KLOs, KHIs, groups = plan
    LA = int(KLOs.sum()) * P
    LB = int(KHIs.sum()) * P
    f32 = mybir.dt.float32
    bf16 = mybir.dt.bfloat16
    i16 = mybir.dt.int16

    import os as _os
    _simclean = _os.environ.get("SIM_CLEAN", "0") == "1"
    nc = bacc.Bacc("TRN2", target_bir_lowering=False, debug=False,
                   dynamic_dma_scratch_size=DMA_SCRATCH)

    hT = nc.dram_tensor("hT", [P, N_NODES_PAD], bf16, kind="ExternalInput")
    hT_own = nc.dram_tensor("hT_own", [P, NODE_PAD], bf16, kind="ExternalInput")
    wcat_d = nc.dram_tensor("wcat", [P, 68], bf16, kind="ExternalInput")
    wad_d = nc.dram_tensor("wad", [P, 4], bf16, kind="ExternalInput")
    gidxA = nc.dram_tensor("gidxA", [P, LA // 16], i16, kind="ExternalInput")
    gidxB = nc.dram_tensor("gidxB", [P, LB // 16], i16, kind="ExternalInput")

    zaug = nc.dram_tensor("zaug", [N_NODES_PAD, 128], bf16)
    outO = nc.dram_tensor("outO", [NODE_PAD, HF], f32, kind="ExternalOutput")

    SC = 4                        # chunks per PSUM tile
    SC2 = 8                       # chunks per load/store superchunk

    with tile.TileContext(nc) as tc:
        with (
            tc.tile_pool(name="const", bufs=1) as cpool,
            tc.tile_pool(name="pa", bufs=8) as pa,
            tc.tile_pool(name="papsum", bufs=4, space="PSUM") as papsum,
            tc.tile_pool(name="adpsum", bufs=1, space="PSUM") as adpsum,
            tc.tile_pool(name="epA", bufs=3) as epA,
            tc.tile_pool(name="epB", bufs=3) as epB,
            tc.tile_pool(name="ep", bufs=4) as ep,
            tc.tile_pool(name="fp", bufs=3) as fp,
        ):
            wcat_t = cpool.tile([P, 68], bf16)
            nc.sync.dma_start(out=wcat_t[:], in_=wcat_d[:])
            gA_t = cpool.tile([P, LA // 16], i16)
            nc.sync.dma_start(out=gA_t[:], in_=gidxA[:])

            # ---------------- Phase A: zaug for all nodes -------------------
            n_sc = NCHUNK // SC2
            all_scs = [(s * SC2, SC2) for s in range(n_sc)]
            if NCHUNK % SC2:
                all_scs.append((n_sc * SC2, NCHUNK % SC2))
            z3 = zaug[:].rearrange("(p s) e -> p s e", s=NCHUNK)
            for s0, nsub in all_scs:
                hsl = pa.tile([P, SC2 * P], bf16, tag="hsl")
                nc.sync.dma_start(
                    out=hsl[:, :nsub * P],
                    in_=hT[:, s0 * P:(s0 + nsub) * P])
                zst = pa.tile([P, SC2, 64], f32, tag="zst")
                if _simclean:
                    nc.scalar.memzero(zst[:])
                zbf = zst[:].bitcast(mybir.dt.bfloat16)
                for g0 in range(0, nsub, SC):
                    g1 = min(g0 + SC, nsub)
                    zp = papsum.tile([P, SC, 68], f32, tag="zp")
                    for j in range(g0, g1):
                        nc.tensor.matmul(
                            out=zp[:, j - g0, :],
                            lhsT=hsl[:, j * P:(j + 1) * P],
                            rhs=wcat_t[:],
                            start=True, stop=True,
                        )
                    nc.scalar.copy(out=zbf[:, g0:g1, 0:64],
                                   in_=zp[:, :g1 - g0, 0:64])
                    nc.scalar.copy(out=zst[:, g0:g1, 32:36],
                                   in_=zp[:, :g1 - g0, 64:68])
                # overwrite as = -60 on the reserved pad rows (p, s) =
                # (0, 0) and (96, 0) before the store
                if s0 == 0:
                    nc.vector.memset(zst[0:1, 0, 32:36], -60.0)
                    nc.vector.memset(zst[96:97, 0, 32:36], -60.0)
                nc.sync.dma_start(out=z3[:, s0:s0 + nsub, :],
                                  in_=zst[:].bitcast(bf16)[:, :nsub, :])

            # ---------------- Phase A0 + remaining loads (overlap edge) -----
            wad_t = cpool.tile([P, 4], bf16)
            nc.sync.dma_start(out=wad_t[:], in_=wad_d[:])
            ho_t = cpool.tile([P, NODE_PAD], bf16)
            nc.sync.dma_start(out=ho_t[:], in_=hT_own[:])
            gB_t = cpool.tile([P, LB // 16], i16)
            nc.sync.dma_start(out=gB_t[:], in_=gidxB[:])
            adp = adpsum.tile([P, BLOCKS, 4], f32)
            for b in range(BLOCKS):
                nc.tensor.matmul(
                    out=adp[:, b, :],
                    lhsT=ho_t[:, b * P:(b + 1) * P],
                    rhs=wad_t[:],
                    start=True, stop=True,
                )
            adall = cpool.tile([P, BLOCKS, 4], f32)
            nc.scalar.copy(out=adall[:], in_=adp[:])

            # ---------------- Edge phase ------------------------------------
            viewA = zaug[0:VIEW, 0:GE]
            viewB = zaug[HIB:N_NODES_PAD, 0:GE]
            outS = cpool.tile([P, BLOCKS, HF], f32)
            o3 = outO[:].rearrange("(p s) e -> p s e", s=BLOCKS)
            offA = np.concatenate([[0], np.cumsum(KLOs)])
            offB = np.concatenate([[0], np.cumsum(KHIs)])
            o16A = o16B = 0
            for g in groups:
                b0, b1 = g[0], g[-1] + 1
                ga_ch = int(offA[b1] - offA[b0])
                gb_ch = int(offB[b1] - offB[b0])
                zsA = epA.tile([P, ga_ch, GE], bf16, tag="zsA")
                for c0 in range(0, ga_ch, CALL_CHUNKS):
                    c1 = min(c0 + CALL_CHUNKS, ga_ch)
                    _gather_narrow(
                        nc, mybir,
                        out_ap=zsA[:, c0:c1, :],
                        in_ap=viewA,
                        idxs_ap=gA_t[:, o16A + c0 * 8:o16A + c1 * 8],
                        num_idxs=(c1 - c0) * P,
                        elem_size=GE, elem_step=128,
                    )
                zsB = epB.tile([P, gb_ch, GE], bf16, tag="zsB")
                for c0 in range(0, gb_ch, CALL_CHUNKS):
                    c1 = min(c0 + CALL_CHUNKS, gb_ch)
                    _gather_narrow(
                        nc, mybir,
                        out_ap=zsB[:, c0:c1, :],
                        in_ap=viewB,
                        idxs_ap=gB_t[:, o16B + c0 * 8:o16B + c1 * 8],
                        num_idxs=(c1 - c0) * P,
                        elem_size=GE, elem_step=128,
                    )
                o16A += ga_ch * 8
                o16B += gb_ch * 8
                for b in g:
                    KA, KB = int(KLOs[b]), int(KHIs[b])
                    K = KA + KB
                    ca = int(offA[b] - offA[b0])
                    cb_ = int(offB[b] - offB[b0])
                    adb = adall[:, b, :].unsqueeze(1)
                    et = ep.tile([P, K, 4], f32, tag="et")
                    nc.vector.tensor_add(
                        out=et[:, 0:KA, :],
                        in0=zsA[:].bitcast(f32)[:, ca:ca + KA, 32:36],
                        in1=adb.broadcast_to([P, KA, 4]))
                    nc.vector.tensor_add(
                        out=et[:, KA:K, :],
                        in0=zsB[:].bitcast(f32)[:, cb_:cb_ + KB, 32:36],
                        in1=adb.broadcast_to([P, KB, 4]))
                    ex1 = ep.tile([P, K, 4], bf16, tag="ex1")
                    nc.scalar.activation(ex1[:], et[:],
                                         mybir.ActivationFunctionType.Exp)
                    ext = ep.tile([P, K, 4], bf16, tag="ext")
                    nc.scalar.activation(ext[:], et[:],
                                         mybir.ActivationFunctionType.Exp,
                                         scale=ALPHA)
                    nc.vector.tensor_tensor(out=ext[:], in0=ext[:], in1=ex1[:],
                                            op=mybir.AluOpType.max)
                    # rhs = [ex * zs | ex]
                    rhs_t = ep.tile([P, K, 68], bf16, tag="rhs")
                    exb = ext[:].unsqueeze(3)
                    zsbA = zsA[:, ca:ca + KA, 0:64].rearrange(
                        "p k (h f) -> p k h f", h=4)
                    nc.vector.tensor_tensor(
                        out=rhs_t[:, 0:KA, 0:64].rearrange(
                            "p k (h f) -> p k h f", h=4),
                        in0=zsbA,
                        in1=exb[:, 0:KA, :, :].broadcast_to([P, KA, 4, 16]),
                        op=mybir.AluOpType.mult)
                    zsbB = zsB[:, cb_:cb_ + KB, 0:64].rearrange(
                        "p k (h f) -> p k h f", h=4)
                    nc.vector.tensor_tensor(
                        out=rhs_t[:, KA:K, 0:64].rearrange(
                            "p k (h f) -> p k h f", h=4),
                        in0=zsbB,
                        in1=exb[:, KA:K, :, :].broadcast_to([P, KB, 4, 16]),
                        op=mybir.AluOpType.mult)
                    nc.scalar.copy(out=rhs_t[:, :, 64:68], in_=ext[:])
                    # [num | denom]: pairwise tree-sum over chunks (all
                    # operands contiguous, f32 accumulation)
                    n = K
                    while n > 1:
                        hh = n // 2
                        nc.vector.tensor_add(
                            out=rhs_t[:, 0:hh, :],
                            in0=rhs_t[:, 0:hh, :],
                            in1=rhs_t[:, n - hh:n, :])
                        n -= hh
                    red = rhs_t[:, 0, :]
                    rec = fp.tile([P, 4], f32, tag="rec")
                    nc.vector.reciprocal(rec[:], red[:, 64:68])
                    nc.vector.tensor_tensor(
                        out=outS[:, b, :].rearrange("p (h f) -> p h f", h=4),
                        in0=red[:, 0:64].rearrange("p (h f) -> p h f", h=4),
                        in1=rec[:].unsqueeze(2).broadcast_to([P, 4, 16]),
                        op=mybir.AluOpType.mult)
                nc.sync.dma_start(out=o3[:, b0:b1, :], in_=outS[:, b0:b1, :])



    nc.finalize()
    return nc


def kernel(h, src, dst, W, A):
    from concourse.bass_utils import run_bass_kernel_spmd

    in_maps, plan, unpack_maps = _build_host_data(h, src, dst, W, A)
    nc = _build_program(plan)
    res = run_bass_kernel_spmd(nc, in_maps, core_ids=list(range(N_CORES)))
    out = np.empty((N_NODES, HF), dtype=np.float32)
    for k in range(N_CORES):
        out[k * NODES_PER_CORE:(k + 1) * NODES_PER_CORE] = \
            res.results[k]["outO"][unpack_maps[k]]
    return out


# revision 46
# speedup vs baseline: 1.3927x; 1.0173x over previous
"""GAT layer (nn_GATLayer) on 8 Trainium2 NeuronCores via Bass/Tile.

Strategy (dst-partitioned; degree-aligned slots, softmax fully local per core):
  - Core k owns dst nodes [k*6250, (k+1)*6250). Each owned node is pinned to a
    (block, partition) slot; ALL of its in-edges occupy that partition across
    the block's chunks. Segment-softmax then needs no scatter at all: the
    per-node sums are free-axis reductions, and the dst attention term ad is a
    per-partition scalar broadcast.
  - Phase A (replicated): zaug[row] = [z bf16 x64 | as f32 x4 | pad] (256B
    rows) for ALL nodes via one matmul with folded weights [W.T | W.T@A1blk].
    Rows are partition-major (row = p*391 + s) so stores are contiguous.
    Two reserved rows (0 and 96*391) get as = -60 written in-tile: pad slots
    gather them and contribute exp(leaky(-60+ad)) ~ 1e-5 to denom and exactly
    0 to num (z = 0).
  - Phase A0 (after phase A, overlapping the edge phase): ad = hT_own @
    (W.T@A2blk) for own nodes in (block, pos) order; SBUF-resident [128,49,4].
  - Edge phase per block: dma_gather of zaug rows by src. The gather moves
    only the useful 144B of each 256B row (elem_size=72 bf16, 256B stride;
    instruction emitted directly since the builder's %256 elem assert is a
    transpose-mode restriction). Two overlapping int16 row views: A = rows
    [0, 32768), B = rows [17280, 50048); the per-node lo/hi edge split is
    chosen on host (min uniform budgets KLO >= max a, KHI >= max c,
    KLO+KHI >= max T per block). e = as + ad; ex = max(exp(e), exp(0.2e));
    rhs = [ex*zs | ex] in bf16; pairwise tree-sum over chunks -> [num|denom];
    out = num * recip(denom); per-group outO stores.
  - Gather calls are capped at 1024 indices (HW SWDGE limit, 994ns desc-gen
    overhead each, Pool is the edge-phase bottleneck). Tile groups of blocks
    are chosen by DP to minimize total call count; the final ~20% use smaller
    groups to shorten the trailing compute tail.

All index/layout prep (row permutation placing high-out-degree nodes in the
overlapping view region, per-core 2D block packing keyed by (a-c, -T//3),
per-block chunk budgets uniformized across cores so one program serves all
8 cores) is done on host in numpy.
"""

import numpy as np

N_NODES = 50000
N_EDGES = 800000
IN_FEATS = 128
OUT_FEATS = 16
NUM_HEADS = 4
ALPHA = 0.2
HF = NUM_HEADS * OUT_FEATS  # 64

N_CORES = 8
P = 128
NODES_PER_CORE = N_NODES // N_CORES     # 6250
BLOCKS = 49                              # ceil(6250/128)
NODE_PAD = BLOCKS * P                    # 6272
NCHUNK = 391                             # zaug chunks; 128*391 = 50048 rows
N_NODES_PAD = P * NCHUNK                 # 50048
VIEW = 32768                             # int16 gather view size
HIB = N_NODES_PAD - VIEW                 # 17280 = base of view B
PAD_A_ROW = 0                            # reserved pad row in view A (p0, s0)
PAD_B_ROW = 96 * NCHUNK                  # reserved pad row in view B (p96, s0)
DMA_SCRATCH = 16384                      # SWDGE ring: 1024 descriptors (HW cap per call)
CALL_CHUNKS = 8                          # max chunks (128 idx each) per gather
TILE_CHUNKS = 64                         # max chunks per stream per zs tile
GE = 72                                  # gather elem: 72 bf16 = 144B of each 256B row


def _wrap16(vals):
    # gather idx layout: stream position i -> idx tile [16, n/16] at
    # [i%16, i//16]; rows replicated to 128 partitions.
    n = vals.shape[-1]
    w = vals.reshape(n // 16, 16).T                    # [16, n/16]
    return np.tile(w, (8, 1))                          # [128, n/16]


def _plan_groups(klos, khis):
    """Partition blocks into consecutive zs-tile groups minimizing the total
    number of gather calls (ceil(sumKLO/CALL) + ceil(sumKHI/CALL) per group),
    groups capped at TILE_CHUNKS per stream (smaller near the end so the
    trailing compute tail stays short)."""
    import math
    nb = len(klos)
    tot = int(klos.sum() + khis.sum())
    pre = np.concatenate([[0], np.cumsum(klos + khis)])
    INF = 10 ** 9

    def cap_at(b):
        return TILE_CHUNKS if pre[b] < 0.7 * tot else TILE_CHUNKS // 3

    # dp[i] = (calls, groups) best for blocks[i:]
    dp = [(INF, INF)] * (nb + 1)
    nxt = [0] * nb
    dp[nb] = (0, 0)
    for i in range(nb - 1, -1, -1):
        sa = sb = 0
        for j in range(i, nb):
            sa += int(klos[j])
            sb += int(khis[j])
            cap = cap_at(i)
            if (j > i) and (sa > cap or sb > cap):
                break
            cost = (math.ceil(sa / CALL_CHUNKS) + math.ceil(sb / CALL_CHUNKS)
                    + dp[j + 1][0], 1 + dp[j + 1][1])
            if cost < dp[i]:
                dp[i] = cost
                nxt[i] = j + 1
    groups = []
    i = 0
    while i < nb:
        groups.append(list(range(i, nxt[i])))
        i = nxt[i]
    return groups


def _build_host_data(h, src, dst, W, A):
    import ml_dtypes

    src = np.asarray(src)
    dst = np.asarray(dst)
    W = np.asarray(W, dtype=np.float32)
    A = np.asarray(A, dtype=np.float32)
    h = np.asarray(h, dtype=np.float32)

    # folded weights
    A1blk = np.zeros((HF, NUM_HEADS), dtype=np.float32)
    A2blk = np.zeros((HF, NUM_HEADS), dtype=np.float32)
    for hd in range(NUM_HEADS):
        A1blk[hd * OUT_FEATS:(hd + 1) * OUT_FEATS, hd] = A[hd, :OUT_FEATS]
        A2blk[hd * OUT_FEATS:(hd + 1) * OUT_FEATS, hd] = A[hd, OUT_FEATS:]
    WT = np.ascontiguousarray(W.T)                                  # [128, 64]
    wcat = np.concatenate([WT, WT @ A1blk], axis=1).astype(ml_dtypes.bfloat16)
    wad = (WT @ A2blk).astype(ml_dtypes.bfloat16)                   # [128, 4]

    # global row permutation: high-out-degree nodes -> overlap rows
    # [HIB, VIEW); rows 0 and 50047 reserved for pad targets.
    outdeg = np.bincount(src, minlength=N_NODES)
    nodes_by_heat = np.argsort(-outdeg, kind="stable")
    ov_rows = np.arange(HIB, VIEW)
    rest_hi = np.arange(VIEW, N_NODES_PAD)
    rest = np.concatenate(
        [np.arange(1, HIB), rest_hi[rest_hi != PAD_B_ROW]])
    perm = np.empty(N_NODES, dtype=np.int64)
    perm[nodes_by_heat[:len(ov_rows)]] = ov_rows
    perm[nodes_by_heat[len(ov_rows):]] = rest[:N_NODES - len(ov_rows)]

    # hT column for row r: phase A chunk s partition p -> row p*391 + s,
    # processed from hT col s*128 + p.
    hT = np.zeros((P, N_NODES_PAD), dtype=ml_dtypes.bfloat16)
    cols = (perm % NCHUNK) * P + perm // NCHUNK
    hT[:, cols] = h.T.astype(ml_dtypes.bfloat16)

    # per-core edge prep
    order = np.argsort(dst, kind="stable")
    dst_s = dst[order]
    rows_s = perm[src[order]]
    core_begin = np.searchsorted(
        dst_s, np.arange(0, N_NODES + 1, NODES_PER_CORE))

    cores = []
    for k in range(N_CORES):
        lo_e, hi_e = core_begin[k], core_begin[k + 1]
        cd = dst_s[lo_e:hi_e] - k * NODES_PER_CORE
        rw = rows_s[lo_e:hi_e]
        is_a = rw < HIB                      # A-only
        is_b = rw >= VIEW                    # B-only
        is_f = ~is_a & ~is_b                 # flexible
        a = np.bincount(cd[is_a], minlength=NODES_PER_CORE)
        c = np.bincount(cd[is_b], minlength=NODES_PER_CORE)
        f = np.bincount(cd[is_f], minlength=NODES_PER_CORE)
        T = a + c + f
        node_order = np.lexsort((a - c, (-T) // 3))
        # node -> (block, pos)
        node_block = np.empty(NODES_PER_CORE, dtype=np.int64)
        node_pos = np.empty(NODES_PER_CORE, dtype=np.int64)
        node_block[node_order] = np.arange(NODES_PER_CORE) // P
        node_pos[node_order] = np.arange(NODES_PER_CORE) % P
        # per-block requirement maxima (budget components)
        amax = np.zeros(BLOCKS, dtype=np.int64)
        cmax = np.zeros(BLOCKS, dtype=np.int64)
        tmax = np.zeros(BLOCKS, dtype=np.int64)
        for b in range(BLOCKS):
            blk = node_order[b * P:(b + 1) * P]
            amax[b] = a[blk].max()
            cmax[b] = c[blk].max()
            tmax[b] = T[blk].max()
        cores.append(dict(cd=cd, rw=rw, a=a, c=c, f=f, T=T,
                          node_block=node_block, node_pos=node_pos,
                          amax=amax, cmax=cmax, tmax=tmax))

    # minimal uniform per-block chunk budgets across cores: any split with
    # KLO >= max a, KHI >= max c, KLO+KHI >= max T is feasible per node.
    Astar = np.max([co["amax"] for co in cores], axis=0)
    Cstar = np.max([co["cmax"] for co in cores], axis=0)
    Tstar = np.max([co["tmax"] for co in cores], axis=0)
    Kb = np.maximum(Tstar, Astar + Cstar)
    KLOs = np.maximum(Astar, Kb - Cstar)
    KHIs = Kb - KLOs
    groups = _plan_groups(KLOs, KHIs)
    LA = int(KLOs.sum()) * P
    LB = int(KHIs.sum()) * P
    offA = np.concatenate([[0], np.cumsum(KLOs)])    # chunk offsets per block
    offB = np.concatenate([[0], np.cumsum(KHIs)])

    in_maps = []
    unpack_maps = []
    for k in range(N_CORES):
        co = cores[k]
        cd, rw = co["cd"], co["rw"]
        a, f, T = co["a"], co["f"], co["T"]
        node_block, node_pos = co["node_block"], co["node_pos"]
        # per-node lo count: L = max(a, T - KHI_block)
        KHI_n = KHIs[node_block]
        L = np.maximum(a, T - KHI_n)

        # sort edges by (node, flexibility-class) so each node's edge list is
        # [A-only..., flex..., B-only...]; first L edges -> stream A.
        cls = np.where(rw < HIB, 0, np.where(rw < VIEW, 1, 2))
        eo = np.lexsort((cls, cd))
        cd_o, rw_o = cd[eo], rw[eo]
        starts = np.searchsorted(cd_o, np.arange(NODES_PER_CORE + 1))
        rank = np.arange(len(cd_o)) - starts[cd_o]          # rank within node
        to_a = rank < L[cd_o]

        gA = np.full((LA // P, P), PAD_A_ROW, dtype=np.int16)
        gB = np.full((LB // P, P), PAD_B_ROW - HIB, dtype=np.int16)
        # slot chunk = offX[block] + rank (A) or rank - L (B)
        blk_e = node_block[cd_o]
        pos_e = node_pos[cd_o]
        ca = offA[blk_e] + rank
        cb_ = offB[blk_e] + rank - L[cd_o]
        gA[ca[to_a], pos_e[to_a]] = rw_o[to_a].astype(np.int16)
        gB[cb_[~to_a], pos_e[~to_a]] = (rw_o[~to_a] - HIB).astype(np.int16)

        # wrap16 per call group
        gAw, gBw = [], []
        for g in groups:
            b0, b1 = g[0], g[-1] + 1
            gAw.append(_wrap16(gA[offA[b0]:offA[b1]].reshape(-1)))
            gBw.append(_wrap16(gB[offB[b0]:offB[b1]].reshape(-1)))
        gAw = np.ascontiguousarray(np.concatenate(gAw, axis=1))
        gBw = np.ascontiguousarray(np.concatenate(gBw, axis=1))

        # hT_own: col b*128 + pos = h[node]
        hT_own = np.zeros((P, NODE_PAD), dtype=ml_dtypes.bfloat16)
        own = np.arange(k * NODES_PER_CORE, (k + 1) * NODES_PER_CORE)
        hT_own[:, node_block * P + node_pos] = h[own].T.astype(
            ml_dtypes.bfloat16)

        in_maps.append({
            "hT": hT,
            "hT_own": hT_own,
            "wcat": np.ascontiguousarray(wcat),
            "wad": np.ascontiguousarray(wad),
            "gidxA": gAw,
            "gidxB": gBw,
        })
        # outO row for node (block, pos) = pos*BLOCKS + block
        unpack_maps.append(node_pos * BLOCKS + node_block)

    return in_maps, (KLOs, KHIs, groups), unpack_maps



def _gather_narrow(nc, mybir, out_ap, in_ap, idxs_ap, num_idxs, elem_size,
                   elem_step):
    """dma_gather with elem_size_bytes not a multiple of 256 (the builder's
    %256 assert is a transpose-mode restriction; non-transpose SDMA
    descriptors support arbitrary lengths). Mirrors BassGpSimd.dma_gather's
    lowering for the plain DRAM-source, gen_mode=0 case."""
    g = nc.gpsimd
    stride_bytes = elem_step * mybir.dt.size(in_ap.dtype)
    _in_ap = g.lower_ap_dma(in_ap, for_custom_bir_dma=True)
    _idxs_ap = g.lower_ap(idxs_ap)
    _out_ap = g.lower_ap(out_ap)
    return g.add_instruction(
        mybir.InstDMAGatherAnt(
            name=nc.get_next_instruction_name(),
            ins=[*_in_ap, _idxs_ap, g.lower_val_access(g.to_reg(num_idxs))],
            outs=[_out_ap],
            transpose=False,
            num_idxs=num_idxs,
            elem_size=elem_size,
            stride_bytes_256=stride_bytes // 256,
            gen_mode=0,
            single_packet=True,
            queue_num=0,
            sbuf_tokens_per_rank=0,
            sbuf_free_dim_per_rank=0,
            sbuf_free_dim_pad_per_rank=0,
            sbuf_byte_offset=0,
        ))


def _build_program(plan):
    import concourse.bacc as bacc
    import concourse.tile as tile
    import concourse.mybir as mybir

    KLOs, KHIs, groups = plan
    LA = int(KLOs.sum()) * P
    LB = int(KHIs.sum()) * P
    f32 = mybir.dt.float32
    bf16 = mybir.dt.bfloat16
    i16 = mybir.dt.int16

    import os as _os
    _simclean = _os.environ.get("SIM_CLEAN", "0") == "1"
    nc = bacc.Bacc("TRN2", target_bir_lowering=False, debug=False,
                   dynamic_dma_scratch_size=DMA_SCRATCH)

    hT = nc.dram_tensor("hT", [P, N_NODES_PAD], bf16, kind="ExternalInput")
    hT_own = nc.dram_tensor("hT_own", [P, NODE_PAD], bf16, kind="ExternalInput")
    wcat_d = nc.dram_tensor("wcat", [P, 68], bf16, kind="ExternalInput")
    wad_d = nc.dram_tensor("wad", [P, 4], bf16, kind="ExternalInput")
    gidxA = nc.dram_tensor("gidxA", [P, LA // 16], i16, kind="ExternalInput")
    gidxB = nc.dram_tensor("gidxB", [P, LB // 16], i16, kind="ExternalInput")

    zaug = nc.dram_tensor("zaug", [N_NODES_PAD, 128], bf16)
    outO = nc.dram_tensor("outO", [NODE_PAD, HF], f32, kind="ExternalOutput")

    SC = 4                        # chunks per PSUM tile
    SC2 = 8                       # chunks per load/store superchunk

    with tile.TileContext(nc) as tc:
        with (
            tc.tile_pool(name="const", bufs=1) as cpool,
            tc.tile_pool(name="pa", bufs=12) as pa,
            tc.tile_pool(name="papsum", bufs=6, space="PSUM") as papsum,
            tc.tile_pool(name="adpsum", bufs=1, space="PSUM") as adpsum,
            tc.tile_pool(name="epA", bufs=3) as epA,
            tc.tile_pool(name="epB", bufs=3) as epB,
            tc.tile_pool(name="ep", bufs=6) as ep,
            tc.tile_pool(name="fp", bufs=3) as fp,
        ):
            wcat_t = cpool.tile([P, 68], bf16)
            nc.sync.dma_start(out=wcat_t[:], in_=wcat_d[:])
            gA_t = cpool.tile([P, LA // 16], i16)
            # only the first groups' indices are needed when gathers start;
            # the rest loads during the edge window
            gacut = (min(int(KLOs[:groups[1][-1] + 1].sum()), LA // P)
                     if len(groups) > 1 else LA // P) * 8
            nc.sync.dma_start(out=gA_t[:, 0:gacut], in_=gidxA[:, 0:gacut])

            # ---------------- Phase A: zaug for all nodes -------------------
            n_sc = NCHUNK // SC2
            all_scs = [(s * SC2, SC2) for s in range(n_sc)]
            if NCHUNK % SC2:
                all_scs.append((n_sc * SC2, NCHUNK % SC2))
            z3 = zaug[:].rearrange("(p s) e -> p s e", s=NCHUNK)
            for s0, nsub in all_scs:
                hsl = pa.tile([P, SC2 * P], bf16, tag="hsl")
                nc.sync.dma_start(
                    out=hsl[:, :nsub * P],
                    in_=hT[:, s0 * P:(s0 + nsub) * P])
                zst = pa.tile([P, SC2, 64], f32, tag="zst")
                if _simclean:
                    nc.scalar.memzero(zst[:])
                zbf = zst[:].bitcast(mybir.dt.bfloat16)
                for g0 in range(0, nsub, SC):
                    g1 = min(g0 + SC, nsub)
                    zp = papsum.tile([P, SC, 68], f32, tag="zp")
                    for j in range(g0, g1):
                        nc.tensor.matmul(
                            out=zp[:, j - g0, :],
                            lhsT=hsl[:, j * P:(j + 1) * P],
                            rhs=wcat_t[:],
                            start=True, stop=True,
                        )
                    nc.scalar.copy(out=zbf[:, g0:g1, 0:64],
                                   in_=zp[:, :g1 - g0, 0:64])
                    nc.scalar.copy(out=zst[:, g0:g1, 32:36],
                                   in_=zp[:, :g1 - g0, 64:68])
                # overwrite as = -60 on the reserved pad rows (p, s) =
                # (0, 0) and (96, 0) before the store
                if s0 == 0:
                    nc.vector.memset(zst[0:1, 0, 32:36], -60.0)
                    nc.vector.memset(zst[96:97, 0, 32:36], -60.0)
                nc.sync.dma_start(out=z3[:, s0:s0 + nsub, :],
                                  in_=zst[:].bitcast(bf16)[:, :nsub, :])

            # ---------------- Phase A0 + remaining loads (overlap edge) -----
            wad_t = cpool.tile([P, 4], bf16)
            nc.sync.dma_start(out=wad_t[:], in_=wad_d[:])
            ho_t = cpool.tile([P, NODE_PAD], bf16)
            nc.sync.dma_start(out=ho_t[:], in_=hT_own[:])
            gB_t = cpool.tile([P, LB // 16], i16)
            nc.sync.dma_start(out=gB_t[:], in_=gidxB[:])
            if gacut < LA // 16:
                nc.sync.dma_start(out=gA_t[:, gacut:], in_=gidxA[:, gacut:])
            adp = adpsum.tile([P, BLOCKS, 4], f32)
            for b in range(BLOCKS):
                nc.tensor.matmul(
                    out=adp[:, b, :],
                    lhsT=ho_t[:, b * P:(b + 1) * P],
                    rhs=wad_t[:],
                    start=True, stop=True,
                )
            adall = cpool.tile([P, BLOCKS, 4], f32)
            nc.scalar.copy(out=adall[:], in_=adp[:])

            # ---------------- Edge phase ------------------------------------
            viewA = zaug[0:VIEW, 0:GE]
            viewB = zaug[HIB:N_NODES_PAD, 0:GE]
            outS = cpool.tile([P, BLOCKS, HF], f32)
            o3 = outO[:].rearrange("(p s) e -> p s e", s=BLOCKS)
            offA = np.concatenate([[0], np.cumsum(KLOs)])
            offB = np.concatenate([[0], np.cumsum(KHIs)])
            o16A = o16B = 0
            for g in groups:
                b0, b1 = g[0], g[-1] + 1
                ga_ch = int(offA[b1] - offA[b0])
                gb_ch = int(offB[b1] - offB[b0])
                zsA = epA.tile([P, ga_ch, GE], bf16, tag="zsA")
                for c0 in range(0, ga_ch, CALL_CHUNKS):
                    c1 = min(c0 + CALL_CHUNKS, ga_ch)
                    _gather_narrow(
                        nc, mybir,
                        out_ap=zsA[:, c0:c1, :],
                        in_ap=viewA,
                        idxs_ap=gA_t[:, o16A + c0 * 8:o16A + c1 * 8],
                        num_idxs=(c1 - c0) * P,
                        elem_size=GE, elem_step=128,
                    )
                zsB = epB.tile([P, gb_ch, GE], bf16, tag="zsB")
                for c0 in range(0, gb_ch, CALL_CHUNKS):
                    c1 = min(c0 + CALL_CHUNKS, gb_ch)
                    _gather_narrow(
                        nc, mybir,
                        out_ap=zsB[:, c0:c1, :],
                        in_ap=viewB,
                        idxs_ap=gB_t[:, o16B + c0 * 8:o16B + c1 * 8],
                        num_idxs=(c1 - c0) * P,
                        elem_size=GE, elem_step=128,
                    )
                o16A += ga_ch * 8
                o16B += gb_ch * 8
                for b in g:
                    KA, KB = int(KLOs[b]), int(KHIs[b])
                    K = KA + KB
                    ca = int(offA[b] - offA[b0])
                    cb_ = int(offB[b] - offB[b0])
                    adb = adall[:, b, :].unsqueeze(1)
                    et = ep.tile([P, K, 4], f32, tag="et")
                    nc.vector.tensor_add(
                        out=et[:, 0:KA, :],
                        in0=zsA[:].bitcast(f32)[:, ca:ca + KA, 32:36],
                        in1=adb.broadcast_to([P, KA, 4]))
                    nc.vector.tensor_add(
                        out=et[:, KA:K, :],
                        in0=zsB[:].bitcast(f32)[:, cb_:cb_ + KB, 32:36],
                        in1=adb.broadcast_to([P, KB, 4]))
                    ex1 = ep.tile([P, K, 4], bf16, tag="ex1")
                    nc.scalar.activation(ex1[:], et[:],
                                         mybir.ActivationFunctionType.Exp)
                    ext = ep.tile([P, K, 4], bf16, tag="ext")
                    nc.scalar.activation(ext[:], et[:],
                                         mybir.ActivationFunctionType.Exp,
                                         scale=ALPHA)
                    nc.vector.tensor_tensor(out=ext[:], in0=ext[:], in1=ex1[:],
                                            op=mybir.AluOpType.max)
                    # rhs = [ex * zs | ex]
                    rhs_t = ep.tile([P, K, 68], bf16, tag="rhs")
                    exb = ext[:].unsqueeze(3)
                    zsbA = zsA[:, ca:ca + KA, 0:64].rearrange(
                        "p k (h f) -> p k h f", h=4)
                    nc.vector.tensor_tensor(
                        out=rhs_t[:, 0:KA, 0:64].rearrange(
                            "p k (h f) -> p k h f", h=4),
                        in0=zsbA,
                        in1=exb[:, 0:KA, :, :].broadcast_to([P, KA, 4, 16]),
                        op=mybir.AluOpType.mult)
                    zsbB = zsB[:, cb_:cb_ + KB, 0:64].rearrange(
                        "p k (h f) -> p k h f", h=4)
                    nc.vector.tensor_tensor(
                        out=rhs_t[:, KA:K, 0:64].rearrange(
                            "p k (h f) -> p k h f", h=4),
                        in0=zsbB,
                        in1=exb[:, KA:K, :, :].broadcast_to([P, KB, 4, 16]),
                        op=mybir.AluOpType.mult)
                    nc.scalar.copy(out=rhs_t[:, :, 64:68], in_=ext[:])
                    # [num | denom]: pairwise tree-sum over chunks (all
                    # operands contiguous, f32 accumulation)
                    n = K
                    while n > 1:
                        hh = n // 2
                        nc.vector.tensor_add(
                            out=rhs_t[:, 0:hh, :],
                            in0=rhs_t[:, 0:hh, :],
                            in1=rhs_t[:, n - hh:n, :])
                        n -= hh
                    red = rhs_t[:, 0, :]
                    rec = fp.tile([P, 4], f32, tag="rec")
                    nc.vector.reciprocal(rec[:], red[:, 64:68])
                    nc.vector.tensor_tensor(
                        out=outS[:, b, :].rearrange("p (h f) -> p h f", h=4),
                        in0=red[:, 0:64].rearrange("p (h f) -> p h f", h=4),
                        in1=rec[:].unsqueeze(2).broadcast_to([P, 4, 16]),
                        op=mybir.AluOpType.mult)
                nc.sync.dma_start(out=o3[:, b0:b1, :], in_=outS[:, b0:b1, :])



    nc.finalize()
    return nc


def kernel(h, src, dst, W, A):
    from concourse.bass_utils import run_bass_kernel_spmd

    in_maps, plan, unpack_maps = _build_host_data(h, src, dst, W, A)
    nc = _build_program(plan)
    res = run_bass_kernel_spmd(nc, in_maps, core_ids=list(range(N_CORES)))
    out = np.empty((N_NODES, HF), dtype=np.float32)
    for k in range(N_CORES):
        out[k * NODES_PER_CORE:(k + 1) * NODES_PER_CORE] = \
            res.results[k]["outO"][unpack_maps[k]]
    return out
